# revision 17
# baseline (speedup 1.0000x reference)
"""Trainium2 Bass kernel for nn_LocalSelfAttention (fused attention block).

Reference (B=2, S=2048, DM=1024, H=16, D=64):
  qkv = x @ Wqkv + bqkv -> split heads -> RoPE(q,k) -> softmax(q k^T/8) v
  -> concat heads @ Wo + bo -> residual + LayerNorm(gamma,beta)

Sharding (8 cores): core c = (batch c//4, query rows 512*(c%4)..+512).
K^T is projected per-core for its OWN 512 positions only and exchanged by
4-way AllGathers per batch replica group; V is recomputed redundantly.
Attention/out-proj/LN are exact and row-local; host gather is pure
concatenation.

v2 changes vs baseline (363.8us):
 * dummy 256B collective issued first: absorbs the ~45us first-collective
   entry barrier under the input DMA loads + K projection.
 * K AllGather split 4 ways (one per head-pair tile) with explicit
   input-side dep edges (the barrier used to order them by accident).
 * emission order K -> Q -> V -> attention: all PSUM-evacuations ride the
   Scalar engine while it is otherwise idle (before the first Exp); the
   Act engine then runs Exp back-to-back with nothing else on its queue.
 * score matmuls emitted hh-alternated: consecutive MMs target row groups
   0-63 / 64-127 so the PE streams both heads' scores concurrently
   (row-group tiling) -- halves the score streaming cycles.
 * softmax rowsum reciprocals in two batches (heads 0-11 after t=5,
   heads 12-15 at the tail); heads 0-11 are normalized mid-attention so
   the out-proj can start immediately at exp-end (kd=6,7 deferred).
 * V bias folded into the residual on host (bv @ Wo term), bc broadcast
   read directly from PSUM by the DVE (no Scalar evac).
"""
import numpy as np
import ml_dtypes

import concourse.bass as bass
import concourse.mybir as mybir
import concourse.tile as tile
from concourse.bass_utils import run_bass_kernel_spmd

BF16 = ml_dtypes.bfloat16
bf16 = mybir.dt.bfloat16
f32 = mybir.dt.float32
AF = mybir.ActivationFunctionType
ALU = mybir.AluOpType
AX = mybir.AxisListType

B, S, DM = 2, 2048, 1024
H, D = 16, 64
NC = 8
ROWS = S * B // NC          # 512 query rows per core
SB = S


# ---- TileContext tail-drain patch: this walrus rejects >1 sync wait on
# CTRL-class instructions; split the global-clock waits onto SP nops.
def _patched_drain_and_barrier(self, tick_clock, wait_clock):
    nc = self.nc
    drain_inst = nc.sync.drain()
    wait_clock.add_sem_waits(
        drain_inst.ins, tile.ScopedClock({None: tick_clock.global_clock})
    )
    si = drain_inst.ins.sync_info
    waits = list(si.on_wait) if si and si.on_wait else []
    if len(waits) > 1:
        si.on_wait = waits[:1]
        for w in waits[1:]:
            nop = nc.sync.nop()
            nop.ins.sync_info = mybir.SyncInfo(on_wait=[w], on_update=[])
    nc.all_engine_barrier()
    assert self.sems is not None
    popped = nc._tile_sem_poison_stack.pop()
    assert popped is self._sem_poison
    nc.all_engine_barrier()


tile.TileContext._drain_and_barrier = _patched_drain_and_barrier

_CTRL_CLASSES = ("InstNoOp", "InstDrain", "InstEventSemaphore")


def _split_excess_waits(nc, maxw_compute=1):
    """Walrus (this version) caps sync waits per instruction (1 for
    CTRL-class, ~2 for compute).  Hoist excess waits onto same-engine NoOps
    inserted immediately before the offending instruction."""
    import copy
    proto = nc.sync.nop().ins  # prototype NoOp (appended to current bb; harmless)
    proto_si = proto.sync_info
    if proto_si and proto_si.on_wait:
        proto.sync_info = mybir.SyncInfo(on_wait=[], on_update=[])
    nsplit = 0
    for f in nc.m.functions:
        for b in f.blocks:
            insts = list(b.instructions)
            out = []
            changed = False
            for inst in insts:
                cls = type(inst).__name__
                maxw = 1 if cls in _CTRL_CLASSES else maxw_compute
                si = inst.sync_info
                waits = list(si.on_wait) if si and si.on_wait else []
                if len(waits) > maxw:
                    keep = waits[:maxw]
                    extra = waits[maxw:]
                    si.on_wait = keep
                    for i, w in enumerate(extra):
                        nop = copy.deepcopy(proto)
                        nop.name = f"{inst.name}-wsplit{i}"
                        nop.engine = inst.engine
                        nop.sync_info = mybir.SyncInfo(on_wait=[w],
                                                       on_update=[])
                        out.append(nop)
                        nsplit += 1
                    changed = True
                out.append(inst)
            if changed:
                try:
                    b.instructions = out
                except Exception:
                    b.set_instructions(out)
    return nsplit


def _build_program():
    nc = bass.Bass("TRN2", target_bir_lowering=False, debug=False,
                   num_devices=NC)

    def din(name, shape, dt):
        return nc.dram_tensor(name, list(shape), dt, kind="ExternalInput").ap()

    xT = din("xT", (DM, SB), bf16)
    xTq = din("xTq", (DM, ROWS), bf16)
    xr = din("xr", (ROWS, DM), f32)          # x rows + bo + bv@Wo (host)
    wq = din("wq", (DM, DM), bf16)
    wk = din("wk", (DM, DM), bf16)
    wv = din("wv", (DM, DM), bf16)
    wo = din("wo", (DM, DM), bf16)
    ccr = din("ccr", (128, ROWS), bf16)
    ssr = din("ssr", (128, ROWS), bf16)
    bqp = din("bqp", (128, 8), f32)
    bkp = din("bkp", (128, 8), f32)
    gbc = din("gbc", (128, DM), bf16)
    bbc = din("bbc", (128, DM), bf16)
    out = nc.dram_tensor("out", [ROWS, DM], f32, kind="ExternalOutput").ap()

    RG = [[0, 1, 2, 3], [4, 5, 6, 7]]

    with tile.TileContext(nc) as tc:
        with tc.tile_pool(name="res", bufs=1) as res, \
             tc.tile_pool(name="tmp", bufs=3) as tmp, \
             tc.tile_pool(name="ppool", bufs=5) as ppool, \
             tc.tile_pool(name="dram", bufs=1, space="DRAM") as dpool:

            # ---- dummy collective: pays the cross-core entry barrier while
            # the input DMAs stream in.  Emitted first on the gpsimd queue.
            dum_in = dpool.tile([1, 128], bf16, name="dum_in")
            dum_out = dpool.tile([4, 128], bf16, name="dum_out")
            nc.gpsimd.collective_compute(
                "AllGather", ALU.bypass, replica_groups=RG,
                ins=[dum_in.opt()], outs=[dum_out.opt()])

            xt_sb = [res.tile([128, SB], bf16, name=f"xt{k}", tag=f"xt{k}") for k in range(8)]
            xq_sb = [res.tile([128, ROWS], bf16, name=f"xq{k}", tag=f"xq{k}") for k in range(8)]
            kT = [res.tile([128, SB], bf16, name=f"kT{t}", tag=f"kT{t}") for t in range(8)]
            qT = [res.tile([128, ROWS], bf16, name=f"qT{t}", tag=f"qT{t}") for t in range(8)]
            vt = [res.tile([128, H * (D + 1)], bf16, name=f"vt{m}", tag=f"vt{m}")
                  for m in range(16)]
            aT = [res.tile([128, ROWS], bf16, name=f"aT{t}", tag=f"aT{t}") for t in range(8)]
            ccr_sb = res.tile([128, ROWS], bf16, tag="ccr")
            ssr_sb = res.tile([128, ROWS], bf16, tag="ssr")
            bq_sb = res.tile([128, 8], f32, tag="bq")
            bk_sb = res.tile([128, 8], f32, tag="bk")
            eps_sb = res.tile([128, 1], f32, tag="eps")

            # load order: K-proj inputs first (wk, xq, rope tables, bias)
            # so the first AllGather triggers as early as possible.
            for k in range(8):
                nc.sync.dma_start(xq_sb[k][:], xTq[k * 128:(k + 1) * 128, :])
            nc.sync.dma_start(ccr_sb[:], ccr[:])
            nc.sync.dma_start(ssr_sb[:], ssr[:])
            nc.sync.dma_start(bk_sb[:], bkp[:])
            nc.sync.dma_start(bq_sb[:], bqp[:])
            nc.vector.memset(eps_sb[:], 1e-5)

            def rope(dst, src, cct, sst, n0, nn):
                # dst[:, n0:n0+nn] = src*CC + swap32(src)*SS
                # (cross-partition 2-input DVE ops are illegal -> copy first)
                t1 = tmp.tile([128, nn], bf16, tag="ropet1")
                t2 = tmp.tile([128, nn], bf16, tag="ropet2")
                for a, b_ in ((0, 32), (32, 0), (64, 96), (96, 64)):
                    nc.vector.tensor_copy(t2[a:a + 32, :], src[b_:b_ + 32, :])
                nc.vector.tensor_tensor(out=t1[:], in0=src[:],
                                        in1=cct[:, n0:n0 + nn], op=ALU.mult)
                nc.vector.tensor_tensor(out=t2[:], in0=t2[:],
                                        in1=sst[:, n0:n0 + nn], op=ALU.mult)
                nc.vector.tensor_tensor(out=dst[:, n0:n0 + nn], in0=t1[:],
                                        in1=t2[:], op=ALU.add)

            # ---- projections ----
            # Each core projects K only for its OWN 512 positions; four
            # 4-way AllGathers (one per head-pair tile pair, per batch
            # replica group) exchange the RoPEd K^T blocks while the PE
            # does Q and the (redundant) V projection.
            with tc.tile_pool(name="wts", bufs=1) as wts, \
                 tc.tile_pool(name="psP", bufs=3, space="PSUM") as psP:
                kin_q = [dpool.tile([256, 512], bf16, name=f"kin_{g}")
                         for g in range(4)]
                kout_q = [dpool.tile([1024, 512], bf16, name=f"kout_{g}")
                          for g in range(4)]

                wk_sb = [wts.tile([128, DM], bf16, name=f"wk{k}", tag=f"wk{k}")
                         for k in range(8)]
                wv_sb = [wts.tile([128, DM], bf16, name=f"wv{k}", tag=f"wv{k}")
                         for k in range(8)]
                kT_own = [wts.tile([128, 512], bf16, name=f"ko{t}",
                                   tag=f"ko{t}") for t in range(8)]
                for k in range(8):
                    nc.sync.dma_start(wk_sb[k][:], wk[k * 128:(k + 1) * 128, :])

                # K^T projection (own 512 positions) + RoPE, then AllGather
                cc_k = [None] * 4
                kin_dmas = []
                for t in range(8):
                    ps = psP.tile([128, 512], f32, tag="proj")
                    for kd in range(8):
                        nc.tensor.matmul(
                            ps[:], wk_sb[kd][:, t * 128:(t + 1) * 128],
                            xq_sb[kd][:], start=(kd == 0), stop=(kd == 7))
                    kt_raw = tmp.tile([128, 512], bf16, tag="evac")
                    nc.scalar.activation(kt_raw[:], ps[:], AF.Identity,
                                         bias=bk_sb[:, t:t + 1])
                    rope(kT_own[t], kt_raw, ccr_sb, ssr_sb, 0, 512)
                    g, tt = t // 2, t % 2
                    # staged from the scalar queue: a sync-queue DMA here
                    # would head-of-line-block the wq/xT/wv loads behind it
                    # while waiting on the RoPE.
                    dma = nc.scalar.dma_start(
                        kin_q[g][tt * 128:(tt + 1) * 128, :], kT_own[t][:])
                    kin_dmas.append(dma)
                    if tt == 1:
                        cc = nc.gpsimd.collective_compute(
                            "AllGather", ALU.bypass, replica_groups=RG,
                            ins=[kin_q[g].opt()], outs=[kout_q[g].opt()])
                        # DRAM tiles are not dependency-tracked: tie the
                        # trigger to the two staging DMAs explicitly.
                        for d_ in kin_dmas[-2:]:
                            bass._add_dep_helper(cc.ins, d_.ins, sync=True,
                                                 reason="AG_K input staged")
                        cc_k[g] = cc

                # Q^T projection + RoPE (wq reuses wk slots)
                wq_sb = [wts.tile([128, DM], bf16, name=f"wq{k}", tag=f"wk{k}")
                         for k in range(8)]
                for k in range(8):
                    nc.sync.dma_start(wq_sb[k][:], wq[k * 128:(k + 1) * 128, :])
                # V-proj inputs stream right behind wq (before the Q-proj
                # emission) so the sync queue never idles the HBM port.
                for k in range(8):
                    nc.sync.dma_start(xt_sb[k][:], xT[k * 128:(k + 1) * 128, :])
                    nc.sync.dma_start(wv_sb[k][:], wv[k * 128:(k + 1) * 128, :])
                for t in range(8):
                    ps = psP.tile([128, 512], f32, tag="proj")
                    for kd in range(8):
                        nc.tensor.matmul(
                            ps[:], wq_sb[kd][:, t * 128:(t + 1) * 128],
                            xq_sb[kd][:], start=(kd == 0), stop=(kd == 7))
                    q_raw = tmp.tile([128, ROWS], bf16, tag="evac")
                    nc.scalar.activation(q_raw[:], ps[:], AF.Identity,
                                         bias=bq_sb[:, t:t + 1])
                    rope(qT[t], q_raw, ccr_sb, ssr_sb, 0, ROWS)

                # V projection (redundant, all 2048 positions; 65-stride
                # head slots + ones column for the softmax rowsums).
                # Scalar-engine evac: Act is idle until the first Exp.
                for m in range(16):
                    m0 = m * 128
                    for ncol in range(2):
                        c0 = ncol * 512
                        ps = psP.tile([128, 512], f32, tag="proj")
                        for kd in range(8):
                            nc.tensor.matmul(
                                ps[:], xt_sb[kd][:, m0:m0 + 128],
                                wv_sb[kd][:, c0:c0 + 512],
                                start=(kd == 0), stop=(kd == 7))
                        dst = vt[m][:, ncol * 8 * 65:(ncol + 1) * 8 * 65]
                        dstv = dst.rearrange("p (h e) -> p h e", e=65)[:, :, 0:64]
                        srcv = ps[:].rearrange("p (h e) -> p h e", e=64)
                        nc.scalar.activation(dstv, srcv, AF.Identity)
                    onev = vt[m][:, :].rearrange("p (h e) -> p h e",
                                                 e=65)[:, :, 64:65]
                    nc.vector.memset(onev, 1.0)

                # gathered K^T -> attention layout (sync queue, after all
                # critical loads; explicit dep edges onto the collectives).
                for g in range(4):
                    for i in range(4):
                        for tt in range(2):
                            t = g * 2 + tt
                            dma = nc.sync.dma_start(
                                kT[t][:, i * 512:(i + 1) * 512],
                                kout_q[g][i * 256 + tt * 128:
                                          i * 256 + (tt + 1) * 128, :])
                            bass._add_dep_helper(dma.ins, cc_k[g].ins,
                                                 sync=True,
                                                 reason="AG_K output read")

            # ---- attention ----
            # scores transposed (S^T = K^T-chunk @ Q^T) into [128,1024] PSUM
            # mega-tiles so each Exp covers FD=1024.  The two heads of a
            # t-pair are emitted alternating so their score MMs stream
            # concurrently through PE row groups 0-63 / 64-127.  PV lags one
            # kc-pair.  Rowsums ride the V ones-column; normalization is
            # deferred and applied to the bf16 aT tiles -- heads 0-11
            # mid-attention (after t=5), heads 12-15 at the tail.
            with tc.tile_pool(name="asb", bufs=1) as asb, \
                 tc.tile_pool(name="psB", bufs=2, space="PSUM") as psB:
                # rowsum of head h lands at partition 32*(h%2), col
                # (h//2)*512 via a (mod-32) partition-shifted DVE copy --
                # no DRAM bounce needed to batch the reciprocals.
                rs_all = asb.tile([33, 4096], bf16, tag="rs_all")
                rinv_all = asb.tile([33, 4096], bf16, tag="rinv_all")
                ones33 = asb.tile([33, 64], bf16, tag="ones33")
                nc.vector.memset(ones33[:], 1.0)

                def rs_pos(h):
                    return 32 * (h % 2), (h // 2) * 512

                def normalize_head(h):
                    # aT[th][po:po+64] *= (1/rowsum_h) broadcast by a PE
                    # outer product reading the reciprocal row in place;
                    # DVE reads the broadcast straight from PSUM (1x mode).
                    th, po = h // 2, 64 * (h % 2)
                    rp, rc = rs_pos(h)
                    bc = psB.tile([128, 512], f32, tag="bc")
                    nc.tensor.matmul(bc[po:po + 64, :],
                                     ones33[rp:rp + 1, :],
                                     rinv_all[rp:rp + 1, rc:rc + 512],
                                     start=True, stop=True)
                    nc.vector.tensor_tensor(out=aT[th][po:po + 64, :],
                                            in0=aT[th][po:po + 64, :],
                                            in1=bc[po:po + 64, :],
                                            op=ALU.mult)

                with tc.tile_pool(name="psA", bufs=2, space="PSUM") as psA, \
                     tc.tile_pool(name="psO", bufs=2, space="PSUM") as psO:

                    for t in range(8):
                        oaccs = [psO.tile([65, 512], f32, tag="oacc",
                                          name=f"oacc{t}_{hh}")
                                 for hh in range(2)]
                        prev = [None, None]

                        def emit_pv(hh, kp, pT_t):
                            h = 2 * t + hh
                            for j in range(2):
                                kc = kp * 2 + j
                                nc.tensor.matmul(
                                    oaccs[hh][:],
                                    vt[kc][:, h * 65:h * 65 + 65],
                                    pT_t[:, j * 512:(j + 1) * 512],
                                    start=(kc == 0), stop=(kc == 15))

                        for kp in range(8):
                            sps = [psA.tile([128, 1024], f32, tag="sco",
                                            name=f"sco{t}_{kp}_{hh}")
                                   for hh in range(2)]
                            # hh-alternated score MMs: row groups 0-63 and
                            # 64-127 stream concurrently.
                            for j in range(2):
                                kc = kp * 2 + j
                                for hh in range(2):
                                    po = 64 * hh
                                    nc.tensor.matmul(
                                        sps[hh][:, j * 512:(j + 1) * 512],
                                        kT[t][po:po + 64,
                                              kc * 128:(kc + 1) * 128],
                                        qT[t][po:po + 64, :],
                                        start=True, stop=True)
                            for hh in range(2):
                                pT = ppool.tile([128, 1024], bf16, tag="pT")
                                nc.scalar.activation(pT[:], sps[hh][:],
                                                     AF.Exp, scale=0.125)
                                if prev[hh] is not None:
                                    emit_pv(hh, kp - 1, prev[hh])
                                prev[hh] = pT
                            # heads 0-11 normalize mid-attention, spread
                            # through t=7's kp slots (PE/DVE slack) so the
                            # out-proj can start the moment Exp finishes.
                            if t == 7 and 2 <= kp:
                                normalize_head(2 * (kp - 2))
                                normalize_head(2 * (kp - 2) + 1)
                        for hh in range(2):
                            emit_pv(hh, 7, prev[hh])
                        # stash rowsum rows (mod-32 partition-shifted
                        # copies) + unnorm. attn (partition shift legal
                        # for 1-input copies)
                        for hh in range(2):
                            h, po = 2 * t + hh, 64 * hh
                            rp, rc = rs_pos(h)
                            nc.vector.tensor_copy(
                                rs_all[rp:rp + 1, rc:rc + 512],
                                oaccs[hh][64:65, :])
                            nc.vector.tensor_copy(aT[t][po:po + 64, :],
                                                  oaccs[hh][0:64, :])
                        # batched rowsum reciprocals: heads 0-11 after t=5
                        # (their aT normalization runs mid-attention under
                        # t=7's kp slots), heads 12-15 at the tail.
                        # Unwritten partition rows compute garbage
                        # reciprocals that are never read.
                        if t in (5, 7):
                            c0, cn = (0, 3072) if t == 5 else (3072, 1024)
                            with nc.allow_low_precision(
                                    reason="softmax 1/rowsum in bf16"):
                                nc.vector.reciprocal(
                                    rinv_all[:, c0:c0 + cn],
                                    rs_all[:, c0:c0 + cn])
                # ---- out-proj + residual + LayerNorm ----
                # (psA/psO closed; psB stays open for heads 12-15.)
                # mr-halves: half 1 (rows 0-255) accumulates kd=0..5 right
                # at exp-end while the heads-12-15 reciprocal chain flows,
                # then kd=6,7 after their normalize; half 2 follows.
                with tc.tile_pool(name="wop", bufs=1) as wop, \
                     tc.tile_pool(name="fin", bufs=2) as fin, \
                     tc.tile_pool(name="psF", bufs=4, space="PSUM") as psF:
                    wo_sb = [wop.tile([128, DM], bf16, name=f"wo{k}",
                                      tag=f"wo{k}") for k in range(8)]
                    for k in range(8):
                        nc.sync.dma_start(wo_sb[k][:],
                                          wo[k * 128:(k + 1) * 128, :])
                    g_sb = wop.tile([128, DM], bf16, tag="g")
                    b_sb = wop.tile([128, DM], bf16, tag="b")
                    nc.sync.dma_start(g_sb[:], gbc[:])
                    nc.sync.dma_start(b_sb[:], bbc[:])

                    def outproj(psf, mrs, kds):
                        for kd in kds:
                            for mr in mrs:
                                rr = mr * 128
                                for ncol in range(2):
                                    c0 = ncol * 512
                                    nc.tensor.matmul(
                                        psf[(mr, ncol)][:],
                                        aT[kd][:, rr:rr + 128],
                                        wo_sb[kd][:, c0:c0 + 512],
                                        start=(kd == 0), stop=(kd == 7))

                    def ln_rows(psf, mr):
                        rr = mr * 128
                        xb = fin.tile([128, DM], f32, tag="xb")
                        nc.sync.dma_start(xb[:], xr[rr:rr + 128, :])
                        # LN arithmetic in bf16: 2x/4x DVE modes; abs error
                        # ~2e-3 of a ~5.0-max output, well inside tolerance
                        hrow = fin.tile([128, DM], bf16, tag="hrow")
                        for ncol in range(2):
                            c0 = ncol * 512
                            nc.vector.tensor_tensor(
                                out=hrow[:, c0:c0 + 512],
                                in0=psf[(mr, ncol)][:],
                                in1=xb[:, c0:c0 + 512], op=ALU.add)
                        ssum = fin.tile([128, 1], f32, tag="ssum")
                        nc.vector.reduce_sum(out=ssum[:], in_=hrow[:],
                                             axis=AX.X)
                        mu = fin.tile([128, 1], f32, tag="mu")
                        nc.vector.tensor_scalar(out=mu[:], in0=ssum[:],
                                                scalar1=1.0 / DM,
                                                scalar2=None, op0=ALU.mult)
                        d = fin.tile([128, DM], bf16, tag="d")
                        nc.vector.tensor_scalar(out=d[:], in0=hrow[:],
                                                scalar1=mu[:], scalar2=None,
                                                op0=ALU.subtract)
                        y = fin.tile([128, DM], bf16, tag="y")
                        vs = fin.tile([128, 1], f32, tag="vs")
                        nc.vector.tensor_tensor(out=y[:], in0=d[:], in1=d[:],
                                                op=ALU.mult)
                        nc.vector.reduce_sum(out=vs[:], in_=y[:], axis=AX.X)
                        st = fin.tile([128, 1], f32, tag="st")
                        nc.scalar.activation(st[:], vs[:], AF.Sqrt,
                                             bias=eps_sb[:], scale=1.0 / DM)
                        rstd = fin.tile([128, 1], f32, tag="rstd")
                        nc.vector.reciprocal(rstd[:], st[:])
                        nc.vector.tensor_scalar(out=y[:], in0=d[:],
                                                scalar1=rstd[:],
                                                scalar2=None, op0=ALU.mult)
                        nc.vector.tensor_tensor(out=y[:], in0=y[:],
                                                in1=g_sb[:], op=ALU.mult)
                        yf = fin.tile([128, DM], f32, tag="yf")
                        nc.vector.tensor_tensor(out=yf[:], in0=y[:],
                                                in1=b_sb[:], op=ALU.add)
                        nc.sync.dma_start(out[rr:rr + 128, :], yf[:])

                    # 4 psF tags x 1 buf = 4 banks; half 2 reuses half 1's
                    # slots (waits on the half-1 LN PSUM reads).
                    psf1 = {(mr, ncol): psF.tile([128, 512], f32, bufs=1,
                                                 tag=f"f{mr}_{ncol}",
                                                 name=f"psfa{mr}_{ncol}")
                            for mr in (0, 1) for ncol in (0, 1)}
                    outproj(psf1, (0, 1), range(6))
                    for h in range(12, 16):
                        normalize_head(h)
                    outproj(psf1, (0, 1), (6, 7))
                    for mr in (0, 1):
                        ln_rows(psf1, mr)
                    # emitted after the half-1 LN reads so the slot-reuse
                    # anti-dependency edges are recorded.
                    psf2 = {(mr, ncol): psF.tile([128, 512], f32, bufs=1,
                                                 tag=f"f{mr - 2}_{ncol}",
                                                 name=f"psfb{mr}_{ncol}")
                            for mr in (2, 3) for ncol in (0, 1)}
                    outproj(psf2, (2, 3), range(8))
                    for mr in (2, 3):
                        ln_rows(psf2, mr)

    _split_excess_waits(nc)
    return nc


_NC_CACHE = None


def _perm():
    p = np.zeros(DM, np.int64)
    for h in range(H):
        p[h * D:h * D + 32] = h * D + np.arange(0, D, 2)
        p[h * D + 32:(h + 1) * D] = h * D + np.arange(1, D, 2)
    return p


def kernel(x, Wqkv, bqkv, Wo, bo, gamma, beta):
    global _NC_CACHE
    x = np.asarray(x, np.float32)
    Wqkv = np.asarray(Wqkv, np.float32)
    bqkv = np.asarray(bqkv, np.float32)
    Wo = np.asarray(Wo, np.float32)
    bo = np.asarray(bo, np.float32)
    gamma = np.asarray(gamma, np.float32)
    beta = np.asarray(beta, np.float32)

    perm = _perm()
    Wq = np.ascontiguousarray(Wqkv[:, 0:DM][:, perm]).astype(BF16)
    Wk = np.ascontiguousarray(Wqkv[:, DM:2 * DM][:, perm]).astype(BF16)
    Wv = np.ascontiguousarray(Wqkv[:, 2 * DM:3 * DM]).astype(BF16)
    Wob = Wo.astype(BF16)
    bq = bqkv[0:DM][perm]
    bk = bqkv[DM:2 * DM][perm]
    bv = bqkv[2 * DM:3 * DM]

    inv = 1.0 / (10000.0 ** (np.arange(0, D, 2, dtype=np.float64) / D))
    pos = np.arange(S, dtype=np.float64)
    fr = pos[None, :] * inv[:, None]                    # [32, S]
    c32, s32 = np.cos(fr), np.sin(fr)
    CC = np.concatenate([c32, c32, c32, c32], 0).astype(BF16)   # [128, S]
    SS = np.concatenate([-s32, s32, -s32, s32], 0).astype(BF16)

    def colmajor(v):
        return np.ascontiguousarray(v.reshape(8, 128).T).astype(np.float32)

    gB = np.ascontiguousarray(np.broadcast_to(gamma, (128, DM))).astype(BF16)
    bB = np.ascontiguousarray(np.broadcast_to(beta, (128, DM))).astype(BF16)
    # bv contributes bv @ Wo to every output row (softmax weights sum to 1);
    # fold it, with bo, into the residual rows on host.
    rbias = bo + bv @ Wo

    if _NC_CACHE is None:
        _NC_CACHE = _build_program()
    nc = _NC_CACHE

    in_maps = []
    for c in range(NC):
        b, r = c // 4, c % 4
        xTb = np.ascontiguousarray(x[b].T).astype(BF16)
        rr = r * ROWS
        in_maps.append({
            "xT": xTb,
            "xTq": np.ascontiguousarray(xTb[:, rr:rr + ROWS]),
            "xr": np.ascontiguousarray(x[b, rr:rr + ROWS, :] + rbias[None, :]),
            "wq": Wq, "wk": Wk, "wv": Wv, "wo": Wob,
            "ccr": np.ascontiguousarray(CC[:, rr:rr + ROWS]),
            "ssr": np.ascontiguousarray(SS[:, rr:rr + ROWS]),
            "bqp": colmajor(bq), "bkp": colmajor(bk),
            "gbc": gB, "bbc": bB,
        })

    res = run_bass_kernel_spmd(nc, in_maps, core_ids=list(range(NC)))
    kernel._last_results = res
    full = np.empty((B, S, DM), np.float32)
    for c in range(NC):
        b, r = c // 4, c % 4
        full[b, r * ROWS:(r + 1) * ROWS, :] = res.results[c]["out"]
    return full


# revision 23
# speedup vs baseline: 1.1607x; 1.1607x over previous
"""Trainium2 Bass kernel for nn_LocalSelfAttention (fused attention block).

Reference (B=2, S=2048, DM=1024, H=16, D=64):
  qkv = x @ Wqkv + bqkv -> split heads -> RoPE(q,k) -> softmax(q k^T/8) v
  -> concat heads @ Wo + bo -> residual + LayerNorm(gamma,beta)

Sharding (8 cores): core c = (batch c//4, query rows 512*(c%4)..+512).
K^T is projected per-core for its OWN 512 positions only and exchanged by
4-way AllGathers per batch replica group; V is recomputed redundantly.
Attention/out-proj/LN are exact and row-local; host gather is pure
concatenation.

v2 changes vs baseline (363.8us):
 * dummy 256B collective issued first: absorbs the ~45us first-collective
   entry barrier under the input DMA loads + K projection.
 * K AllGather split 4 ways (one per head-pair tile) with explicit
   input-side dep edges (the barrier used to order them by accident).
 * emission order K -> Q -> V -> attention: all PSUM-evacuations ride the
   Scalar engine while it is otherwise idle (before the first Exp); the
   Act engine then runs Exp back-to-back with nothing else on its queue.
 * score matmuls emitted hh-alternated: consecutive MMs target row groups
   0-63 / 64-127 so the PE streams both heads' scores concurrently
   (row-group tiling) -- halves the score streaming cycles.
 * softmax rowsum reciprocals in two batches (heads 0-11 after t=5,
   heads 12-15 at the tail); heads 0-11 are normalized mid-attention so
   the out-proj can start immediately at exp-end (kd=6,7 deferred).
 * V bias folded into the residual on host (bv @ Wo term), bc broadcast
   read directly from PSUM by the DVE (no Scalar evac).
"""
import numpy as np
import ml_dtypes

import concourse.bass as bass
import concourse.mybir as mybir
import concourse.tile as tile
from concourse.bass_utils import run_bass_kernel_spmd

BF16 = ml_dtypes.bfloat16
bf16 = mybir.dt.bfloat16
f32 = mybir.dt.float32
AF = mybir.ActivationFunctionType
ALU = mybir.AluOpType
AX = mybir.AxisListType

B, S, DM = 2, 2048, 1024
H, D = 16, 64
NC = 8
ROWS = S * B // NC          # 512 query rows per core
SB = S


# ---- TileContext tail-drain patch: this walrus rejects >1 sync wait on
# CTRL-class instructions; split the global-clock waits onto SP nops.
def _patched_drain_and_barrier(self, tick_clock, wait_clock):
    nc = self.nc
    drain_inst = nc.sync.drain()
    wait_clock.add_sem_waits(
        drain_inst.ins, tile.ScopedClock({None: tick_clock.global_clock})
    )
    si = drain_inst.ins.sync_info
    waits = list(si.on_wait) if si and si.on_wait else []
    if len(waits) > 1:
        si.on_wait = waits[:1]
        for w in waits[1:]:
            nop = nc.sync.nop()
            nop.ins.sync_info = mybir.SyncInfo(on_wait=[w], on_update=[])
    nc.all_engine_barrier()
    assert self.sems is not None
    popped = nc._tile_sem_poison_stack.pop()
    assert popped is self._sem_poison
    nc.all_engine_barrier()


tile.TileContext._drain_and_barrier = _patched_drain_and_barrier

_CTRL_CLASSES = ("InstNoOp", "InstDrain", "InstEventSemaphore")


def _split_excess_waits(nc, maxw_compute=1):
    """Walrus (this version) caps sync waits per instruction (1 for
    CTRL-class, ~2 for compute).  Hoist excess waits onto same-engine NoOps
    inserted immediately before the offending instruction."""
    import copy
    proto = nc.sync.nop().ins  # prototype NoOp (appended to current bb; harmless)
    proto_si = proto.sync_info
    if proto_si and proto_si.on_wait:
        proto.sync_info = mybir.SyncInfo(on_wait=[], on_update=[])
    nsplit = 0
    for f in nc.m.functions:
        for b in f.blocks:
            insts = list(b.instructions)
            out = []
            changed = False
            for inst in insts:
                cls = type(inst).__name__
                maxw = 1 if cls in _CTRL_CLASSES else maxw_compute
                si = inst.sync_info
                waits = list(si.on_wait) if si and si.on_wait else []
                if len(waits) > maxw:
                    keep = waits[:maxw]
                    extra = waits[maxw:]
                    si.on_wait = keep
                    for i, w in enumerate(extra):
                        nop = copy.deepcopy(proto)
                        nop.name = f"{inst.name}-wsplit{i}"
                        nop.engine = inst.engine
                        nop.sync_info = mybir.SyncInfo(on_wait=[w],
                                                       on_update=[])
                        out.append(nop)
                        nsplit += 1
                    changed = True
                out.append(inst)
            if changed:
                try:
                    b.instructions = out
                except Exception:
                    b.set_instructions(out)
    return nsplit


def _build_program():
    nc = bass.Bass("TRN2", target_bir_lowering=False, debug=False,
                   num_devices=NC)

    def din(name, shape, dt):
        return nc.dram_tensor(name, list(shape), dt, kind="ExternalInput").ap()

    xT = din("xT", (DM, SB), bf16)
    xTq = din("xTq", (DM, ROWS), bf16)
    xr = din("xr", (ROWS, DM), f32)          # x rows + bo + bv@Wo (host)
    wq = din("wq", (DM, DM), bf16)
    wk = din("wk", (DM, DM), bf16)
    wv = din("wv", (DM, DM), bf16)
    wo = din("wo", (DM, DM), bf16)
    ccr = din("ccr", (128, ROWS), bf16)
    ssr = din("ssr", (128, ROWS), bf16)
    bqp = din("bqp", (128, 8), f32)
    bkp = din("bkp", (128, 8), f32)
    gbc = din("gbc", (128, DM), bf16)
    bbc = din("bbc", (128, DM), bf16)
    out = nc.dram_tensor("out", [ROWS, DM], f32, kind="ExternalOutput").ap()

    RG = [[0, 1, 2, 3], [4, 5, 6, 7]]

    with tile.TileContext(nc) as tc:
        with tc.tile_pool(name="res", bufs=1) as res, \
             tc.tile_pool(name="tmp", bufs=3) as tmp, \
             tc.tile_pool(name="ppool", bufs=5) as ppool, \
             tc.tile_pool(name="dram", bufs=1, space="DRAM") as dpool:

            xt_sb = [res.tile([128, SB], bf16, name=f"xt{k}", tag=f"xt{k}") for k in range(8)]
            xq_sb = [res.tile([128, ROWS], bf16, name=f"xq{k}", tag=f"xq{k}") for k in range(8)]
            kT = [res.tile([128, SB], bf16, name=f"kT{t}", tag=f"kT{t}") for t in range(8)]
            qT = [res.tile([128, ROWS], bf16, name=f"qT{t}", tag=f"qT{t}") for t in range(8)]
            vt = [res.tile([128, H * (D + 1)], bf16, name=f"vt{m}", tag=f"vt{m}")
                  for m in range(16)]
            aT = [res.tile([128, ROWS], bf16, name=f"aT{t}", tag=f"aT{t}") for t in range(8)]
            ccr_sb = res.tile([128, ROWS], bf16, tag="ccr")
            ssr_sb = res.tile([128, ROWS], bf16, tag="ssr")
            bq_sb = res.tile([128, 8], f32, tag="bq")
            bk_sb = res.tile([128, 8], f32, tag="bk")
            eps_sb = res.tile([128, 1], f32, tag="eps")

            # load order: K-proj inputs first (wk, xq, rope tables, bias)
            # so the first AllGather triggers as early as possible.  The
            # V-proj x^T tiles ride the scalar queue's DMA engine in
            # parallel with the sync-queue loads.
            for k in range(8):
                nc.sync.dma_start(xq_sb[k][:], xTq[k * 128:(k + 1) * 128, :])
                nc.scalar.dma_start(xt_sb[k][:], xT[k * 128:(k + 1) * 128, :])
            nc.sync.dma_start(ccr_sb[:], ccr[:])
            nc.sync.dma_start(ssr_sb[:], ssr[:])
            nc.sync.dma_start(bk_sb[:], bkp[:])
            nc.sync.dma_start(bq_sb[:], bqp[:])
            nc.vector.memset(eps_sb[:], 1e-5)

            def rope(dst, src, cct, sst, n0, nn):
                # dst[:, n0:n0+nn] = src*CC + swap32(src)*SS
                # (cross-partition 2-input DVE ops are illegal -> copy first)
                t1 = tmp.tile([128, nn], bf16, tag="ropet1")
                t2 = tmp.tile([128, nn], bf16, tag="ropet2")
                for a, b_ in ((0, 32), (32, 0), (64, 96), (96, 64)):
                    nc.vector.tensor_copy(t2[a:a + 32, :], src[b_:b_ + 32, :])
                nc.vector.tensor_tensor(out=t1[:], in0=src[:],
                                        in1=cct[:, n0:n0 + nn], op=ALU.mult)
                nc.vector.tensor_tensor(out=t2[:], in0=t2[:],
                                        in1=sst[:, n0:n0 + nn], op=ALU.mult)
                nc.vector.tensor_tensor(out=dst[:, n0:n0 + nn], in0=t1[:],
                                        in1=t2[:], op=ALU.add)

            # ---- projections ----
            # Each core projects K only for its OWN 512 positions; four
            # 4-way AllGathers (one per head-pair tile pair, per batch
            # replica group) exchange the RoPEd K^T blocks while the PE
            # does Q and the (redundant) V projection.
            with tc.tile_pool(name="wts", bufs=1) as wts, \
                 tc.tile_pool(name="psP", bufs=3, space="PSUM") as psP:
                kin_q = [dpool.tile([256, 512], bf16, name=f"kin_{g}")
                         for g in range(4)]
                kout_q = [dpool.tile([1024, 512], bf16, name=f"kout_{g}")
                          for g in range(4)]

                wk_sb = [wts.tile([128, DM], bf16, name=f"wk{k}", tag=f"wk{k}")
                         for k in range(8)]
                wv_sb = [wts.tile([128, DM], bf16, name=f"wv{k}", tag=f"wv{k}")
                         for k in range(8)]
                kT_own = [wts.tile([128, 512], bf16, name=f"ko{t}",
                                   tag=f"ko{t}") for t in range(8)]
                for k in range(8):
                    nc.sync.dma_start(wk_sb[k][:], wk[k * 128:(k + 1) * 128, :])

                # K^T projection (own 512 positions) + RoPE, then AllGather
                cc_k = [None] * 4
                kin_dmas = []
                for t in range(8):
                    ps = psP.tile([128, 512], f32, tag="proj")
                    for kd in range(8):
                        nc.tensor.matmul(
                            ps[:], wk_sb[kd][:, t * 128:(t + 1) * 128],
                            xq_sb[kd][:], start=(kd == 0), stop=(kd == 7))
                    kt_raw = tmp.tile([128, 512], bf16, tag="evac")
                    nc.scalar.activation(kt_raw[:], ps[:], AF.Identity,
                                         bias=bk_sb[:, t:t + 1])
                    rope(kT_own[t], kt_raw, ccr_sb, ssr_sb, 0, 512)
                    g, tt = t // 2, t % 2
                    # staged from the scalar queue: a sync-queue DMA here
                    # would head-of-line-block the wq/xT/wv loads behind it
                    # while waiting on the RoPE.
                    dma = nc.scalar.dma_start(
                        kin_q[g][tt * 128:(tt + 1) * 128, :], kT_own[t][:])
                    kin_dmas.append(dma)
                    if tt == 1:
                        cc = nc.gpsimd.collective_compute(
                            "AllGather", ALU.bypass, replica_groups=RG,
                            ins=[kin_q[g].opt()], outs=[kout_q[g].opt()])
                        # DRAM tiles are not dependency-tracked: tie the
                        # trigger to the two staging DMAs explicitly.
                        for d_ in kin_dmas[-2:]:
                            bass._add_dep_helper(cc.ins, d_.ins, sync=True,
                                                 reason="AG_K input staged")
                        cc_k[g] = cc

                # Q^T projection + RoPE (wq reuses wk slots)
                wq_sb = [wts.tile([128, DM], bf16, name=f"wq{k}", tag=f"wk{k}")
                         for k in range(8)]
                for k in range(8):
                    nc.sync.dma_start(wq_sb[k][:], wq[k * 128:(k + 1) * 128, :])
                # wv streams right behind wq (before the Q-proj emission)
                # so the sync queue never idles the HBM port.
                for k in range(8):
                    nc.sync.dma_start(wv_sb[k][:], wv[k * 128:(k + 1) * 128, :])
                for t in range(8):
                    ps = psP.tile([128, 512], f32, tag="proj")
                    for kd in range(8):
                        nc.tensor.matmul(
                            ps[:], wq_sb[kd][:, t * 128:(t + 1) * 128],
                            xq_sb[kd][:], start=(kd == 0), stop=(kd == 7))
                    q_raw = tmp.tile([128, ROWS], bf16, tag="evac")
                    nc.scalar.activation(q_raw[:], ps[:], AF.Identity,
                                         bias=bq_sb[:, t:t + 1])
                    rope(qT[t], q_raw, ccr_sb, ssr_sb, 0, ROWS)

                # V projection (redundant, all 2048 positions; 65-stride
                # head slots + ones column for the softmax rowsums).
                # Scalar-engine evac: Act is idle until the first Exp.
                for m in range(16):
                    m0 = m * 128
                    for ncol in range(2):
                        c0 = ncol * 512
                        ps = psP.tile([128, 512], f32, tag="proj")
                        for kd in range(8):
                            nc.tensor.matmul(
                                ps[:], xt_sb[kd][:, m0:m0 + 128],
                                wv_sb[kd][:, c0:c0 + 512],
                                start=(kd == 0), stop=(kd == 7))
                        dst = vt[m][:, ncol * 8 * 65:(ncol + 1) * 8 * 65]
                        dstv = dst.rearrange("p (h e) -> p h e", e=65)[:, :, 0:64]
                        srcv = ps[:].rearrange("p (h e) -> p h e", e=64)
                        nc.scalar.activation(dstv, srcv, AF.Identity)
                    onev = vt[m][:, :].rearrange("p (h e) -> p h e",
                                                 e=65)[:, :, 64:65]
                    nc.vector.memset(onev, 1.0)

                # gathered K^T -> attention layout (sync queue, after all
                # critical loads; explicit dep edges onto the collectives).
                for g in range(4):
                    for i in range(4):
                        for tt in range(2):
                            t = g * 2 + tt
                            dma = nc.sync.dma_start(
                                kT[t][:, i * 512:(i + 1) * 512],
                                kout_q[g][i * 256 + tt * 128:
                                          i * 256 + (tt + 1) * 128, :])
                            bass._add_dep_helper(dma.ins, cc_k[g].ins,
                                                 sync=True,
                                                 reason="AG_K output read")

            # ---- attention ----
            # scores transposed (S^T = K^T-chunk @ Q^T) into [128,1024] PSUM
            # mega-tiles so each Exp covers FD=1024.  The two heads of a
            # t-pair are emitted alternating so their score MMs stream
            # concurrently through PE row groups 0-63 / 64-127.  PV lags one
            # kc-pair.  Rowsums ride the V ones-column; normalization is
            # deferred and applied to the bf16 aT tiles -- heads 0-11
            # mid-attention (after t=5), heads 12-15 at the tail.
            with tc.tile_pool(name="asb", bufs=1) as asb:
                # rowsum of head h lands at partition 32*(h%2), col
                # (h//2)*512 via a (mod-32) partition-shifted DVE copy --
                # no DRAM bounce needed to batch the reciprocals.
                rs_all = asb.tile([33, 4096], bf16, tag="rs_all")
                rinv_all = asb.tile([33, 4096], bf16, tag="rinv_all")
                ones33 = asb.tile([33, 64], bf16, tag="ones33")
                nc.vector.memset(ones33[:], 1.0)

                def rs_pos(h):
                    return 32 * (h % 2), (h // 2) * 512

                with tc.tile_pool(name="psA", bufs=3, space="PSUM") as psA, \
                     tc.tile_pool(name="psO", bufs=2, space="PSUM") as psO:

                    for t in range(8):
                        oaccs = [psO.tile([65, 512], f32, tag="oacc",
                                          name=f"oacc{t}_{hh}")
                                 for hh in range(2)]
                        prev = [None, None]

                        def emit_pv(hh, kp, pT_t):
                            h = 2 * t + hh
                            for j in range(2):
                                kc = kp * 2 + j
                                nc.tensor.matmul(
                                    oaccs[hh][:],
                                    vt[kc][:, h * 65:h * 65 + 65],
                                    pT_t[:, j * 512:(j + 1) * 512],
                                    start=(kc == 0), stop=(kc == 15))

                        for kp in range(8):
                            sps = [psA.tile([128, 1024], f32, tag="sco",
                                            name=f"sco{t}_{kp}_{hh}")
                                   for hh in range(2)]
                            # hh-alternated score MMs: row groups 0-63 and
                            # 64-127 stream concurrently.
                            for j in range(2):
                                kc = kp * 2 + j
                                for hh in range(2):
                                    po = 64 * hh
                                    nc.tensor.matmul(
                                        sps[hh][:, j * 512:(j + 1) * 512],
                                        kT[t][po:po + 64,
                                              kc * 128:(kc + 1) * 128],
                                        qT[t][po:po + 64, :],
                                        start=True, stop=True)
                            for hh in range(2):
                                pT = ppool.tile([128, 1024], bf16, tag="pT")
                                nc.scalar.activation(pT[:], sps[hh][:],
                                                     AF.Exp, scale=0.125)
                                if prev[hh] is not None:
                                    emit_pv(hh, kp - 1, prev[hh])
                                prev[hh] = pT
                        for hh in range(2):
                            emit_pv(hh, 7, prev[hh])
                        # stash rowsum rows (mod-32 partition-shifted
                        # copies) + unnorm. attn (partition shift legal
                        # for 1-input copies)
                        for hh in range(2):
                            h, po = 2 * t + hh, 64 * hh
                            rp, rc = rs_pos(h)
                            nc.vector.tensor_copy(
                                rs_all[rp:rp + 1, rc:rc + 512],
                                oaccs[hh][64:65, :])
                            nc.vector.tensor_copy(aT[t][po:po + 64, :],
                                                  oaccs[hh][0:64, :])
                        # batched rowsum reciprocals: heads 0-13 after
                        # t=6 (the big DVE op rides t=7's exp window,
                        # off the aT-copy critical path), heads 14-15 at
                        # the tail.  Unwritten partition rows compute
                        # garbage reciprocals that are never read.
                        if t in (6, 7):
                            c0, cn = (0, 3584) if t == 6 else (3584, 512)
                            with nc.allow_low_precision(
                                    reason="softmax 1/rowsum in bf16"):
                                nc.vector.reciprocal(
                                    rinv_all[:, c0:c0 + cn],
                                    rs_all[:, c0:c0 + cn])
                # ---- out-proj + residual + LayerNorm ----
                # (psA/psO closed -> PSUM free for psF + psB.)
                # All 16 head-normalizes run here, interleaved with the
                # out-proj kd accumulation so the PE never waits: head
                # pair 2t,2t+1 normalizes, then kd=t accumulates.
                with tc.tile_pool(name="wop", bufs=1) as wop, \
                     tc.tile_pool(name="fin", bufs=2) as fin, \
                     tc.tile_pool(name="psB", bufs=2, space="PSUM") as psB, \
                     tc.tile_pool(name="psF", bufs=4, space="PSUM") as psF:

                    def normalize_head(h):
                        # aT[th][po:po+64] *= (1/rowsum_h) broadcast by a
                        # PE outer product reading the reciprocal row in
                        # place; DVE reads the broadcast straight from
                        # PSUM (1x mode).
                        th, po = h // 2, 64 * (h % 2)
                        rp, rc = rs_pos(h)
                        bc = psB.tile([128, 512], f32, tag="bc")
                        nc.tensor.matmul(bc[po:po + 64, :],
                                         ones33[rp:rp + 1, :],
                                         rinv_all[rp:rp + 1, rc:rc + 512],
                                         start=True, stop=True)
                        nc.vector.tensor_tensor(out=aT[th][po:po + 64, :],
                                                in0=aT[th][po:po + 64, :],
                                                in1=bc[po:po + 64, :],
                                                op=ALU.mult)
                    wo_sb = [wop.tile([128, DM], bf16, name=f"wo{k}",
                                      tag=f"wo{k}") for k in range(8)]
                    for k in range(8):
                        nc.sync.dma_start(wo_sb[k][:],
                                          wo[k * 128:(k + 1) * 128, :])
                    g_sb = wop.tile([128, DM], bf16, tag="g")
                    b_sb = wop.tile([128, DM], bf16, tag="b")
                    nc.sync.dma_start(g_sb[:], gbc[:])
                    nc.sync.dma_start(b_sb[:], bbc[:])

                    def outproj(psf, mrs, kds):
                        for kd in kds:
                            for mr in mrs:
                                rr = mr * 128
                                for ncol in range(2):
                                    c0 = ncol * 512
                                    nc.tensor.matmul(
                                        psf[(mr, ncol)][:],
                                        aT[kd][:, rr:rr + 128],
                                        wo_sb[kd][:, c0:c0 + 512],
                                        start=(kd == 0), stop=(kd == 7))

                    def ln_rows(psf, mr):
                        rr = mr * 128
                        xb = fin.tile([128, DM], f32, tag="xb")
                        nc.sync.dma_start(xb[:], xr[rr:rr + 128, :])
                        # LN arithmetic in bf16: 2x/4x DVE modes; abs error
                        # ~2e-3 of a ~5.0-max output, well inside tolerance
                        hrow = fin.tile([128, DM], bf16, tag="hrow")
                        for ncol in range(2):
                            c0 = ncol * 512
                            nc.vector.tensor_tensor(
                                out=hrow[:, c0:c0 + 512],
                                in0=psf[(mr, ncol)][:],
                                in1=xb[:, c0:c0 + 512], op=ALU.add)
                        ssum = fin.tile([128, 1], f32, tag="ssum")
                        nc.vector.reduce_sum(out=ssum[:], in_=hrow[:],
                                             axis=AX.X)
                        mu = fin.tile([128, 1], f32, tag="mu")
                        nc.vector.tensor_scalar(out=mu[:], in0=ssum[:],
                                                scalar1=1.0 / DM,
                                                scalar2=None, op0=ALU.mult)
                        d = fin.tile([128, DM], bf16, tag="d")
                        nc.vector.tensor_scalar(out=d[:], in0=hrow[:],
                                                scalar1=mu[:], scalar2=None,
                                                op0=ALU.subtract)
                        y = fin.tile([128, DM], bf16, tag="y")
                        vs = fin.tile([128, 1], f32, tag="vs")
                        nc.vector.tensor_tensor(out=y[:], in0=d[:], in1=d[:],
                                                op=ALU.mult)
                        nc.vector.reduce_sum(out=vs[:], in_=y[:], axis=AX.X)
                        st = fin.tile([128, 1], f32, tag="st")
                        nc.scalar.activation(st[:], vs[:], AF.Sqrt,
                                             bias=eps_sb[:], scale=1.0 / DM)
                        rstd = fin.tile([128, 1], f32, tag="rstd")
                        nc.vector.reciprocal(rstd[:], st[:])
                        nc.vector.tensor_scalar(out=y[:], in0=d[:],
                                                scalar1=rstd[:],
                                                scalar2=None, op0=ALU.mult)
                        nc.vector.tensor_tensor(out=y[:], in0=y[:],
                                                in1=g_sb[:], op=ALU.mult)
                        yf = fin.tile([128, DM], f32, tag="yf")
                        nc.vector.tensor_tensor(out=yf[:], in0=y[:],
                                                in1=b_sb[:], op=ALU.add)
                        nc.sync.dma_start(out[rr:rr + 128, :], yf[:])

                    # 4 psF tags x 1 buf = 4 banks; half 2 reuses half 1's
                    # slots (waits on the half-1 LN PSUM reads).
                    psf1 = {(mr, ncol): psF.tile([128, 512], f32, bufs=1,
                                                 tag=f"f{mr}_{ncol}",
                                                 name=f"psfa{mr}_{ncol}")
                            for mr in (0, 1) for ncol in (0, 1)}
                    for t in range(8):
                        normalize_head(2 * t)
                        normalize_head(2 * t + 1)
                        outproj(psf1, (0, 1), (t,))
                    for mr in (0, 1):
                        ln_rows(psf1, mr)
                    # emitted after the half-1 LN reads so the slot-reuse
                    # anti-dependency edges are recorded.
                    psf2 = {(mr, ncol): psF.tile([128, 512], f32, bufs=1,
                                                 tag=f"f{mr - 2}_{ncol}",
                                                 name=f"psfb{mr}_{ncol}")
                            for mr in (2, 3) for ncol in (0, 1)}
                    outproj(psf2, (2, 3), range(8))
                    for mr in (2, 3):
                        ln_rows(psf2, mr)

    _split_excess_waits(nc)
    return nc


_NC_CACHE = None


def _perm():
    p = np.zeros(DM, np.int64)
    for h in range(H):
        p[h * D:h * D + 32] = h * D + np.arange(0, D, 2)
        p[h * D + 32:(h + 1) * D] = h * D + np.arange(1, D, 2)
    return p


def kernel(x, Wqkv, bqkv, Wo, bo, gamma, beta):
    global _NC_CACHE
    x = np.asarray(x, np.float32)
    Wqkv = np.asarray(Wqkv, np.float32)
    bqkv = np.asarray(bqkv, np.float32)
    Wo = np.asarray(Wo, np.float32)
    bo = np.asarray(bo, np.float32)
    gamma = np.asarray(gamma, np.float32)
    beta = np.asarray(beta, np.float32)

    perm = _perm()
    Wq = np.ascontiguousarray(Wqkv[:, 0:DM][:, perm]).astype(BF16)
    Wk = np.ascontiguousarray(Wqkv[:, DM:2 * DM][:, perm]).astype(BF16)
    Wv = np.ascontiguousarray(Wqkv[:, 2 * DM:3 * DM]).astype(BF16)
    Wob = Wo.astype(BF16)
    bq = bqkv[0:DM][perm]
    bk = bqkv[DM:2 * DM][perm]
    bv = bqkv[2 * DM:3 * DM]

    inv = 1.0 / (10000.0 ** (np.arange(0, D, 2, dtype=np.float64) / D))
    pos = np.arange(S, dtype=np.float64)
    fr = pos[None, :] * inv[:, None]                    # [32, S]
    c32, s32 = np.cos(fr), np.sin(fr)
    CC = np.concatenate([c32, c32, c32, c32], 0).astype(BF16)   # [128, S]
    SS = np.concatenate([-s32, s32, -s32, s32], 0).astype(BF16)

    def colmajor(v):
        return np.ascontiguousarray(v.reshape(8, 128).T).astype(np.float32)

    gB = np.ascontiguousarray(np.broadcast_to(gamma, (128, DM))).astype(BF16)
    bB = np.ascontiguousarray(np.broadcast_to(beta, (128, DM))).astype(BF16)
    # bv contributes bv @ Wo to every output row (softmax weights sum to 1);
    # fold it, with bo, into the residual rows on host.
    rbias = bo + bv @ Wo

    if _NC_CACHE is None:
        _NC_CACHE = _build_program()
    nc = _NC_CACHE

    in_maps = []
    for c in range(NC):
        b, r = c // 4, c % 4
        xTb = np.ascontiguousarray(x[b].T).astype(BF16)
        rr = r * ROWS
        in_maps.append({
            "xT": xTb,
            "xTq": np.ascontiguousarray(xTb[:, rr:rr + ROWS]),
            "xr": np.ascontiguousarray(x[b, rr:rr + ROWS, :] + rbias[None, :]),
            "wq": Wq, "wk": Wk, "wv": Wv, "wo": Wob,
            "ccr": np.ascontiguousarray(CC[:, rr:rr + ROWS]),
            "ssr": np.ascontiguousarray(SS[:, rr:rr + ROWS]),
            "bqp": colmajor(bq), "bkp": colmajor(bk),
            "gbc": gB, "bbc": bB,
        })

    res = run_bass_kernel_spmd(nc, in_maps, core_ids=list(range(NC)))
    kernel._last_results = res
    full = np.empty((B, S, DM), np.float32)
    for c in range(NC):
        b, r = c // 4, c % 4
        full[b, r * ROWS:(r + 1) * ROWS, :] = res.results[c]["out"]
    return full


# revision 24
# speedup vs baseline: 1.2300x; 1.0597x over previous
"""Trainium2 Bass kernel for nn_LocalSelfAttention (fused attention block).

Reference (B=2, S=2048, DM=1024, H=16, D=64):
  qkv = x @ Wqkv + bqkv -> split heads -> RoPE(q,k) -> softmax(q k^T/8) v
  -> concat heads @ Wo + bo -> residual + LayerNorm(gamma,beta)

Sharding (8 cores): core c = (batch c//4, query rows 512*(c%4)..+512).
K^T is projected per-core for its OWN 512 positions only and exchanged by
4-way AllGathers per batch replica group; V is recomputed redundantly.
Attention/out-proj/LN are exact and row-local; host gather is pure
concatenation.

v2 changes vs baseline (363.8us):
 * dummy 256B collective issued first: absorbs the ~45us first-collective
   entry barrier under the input DMA loads + K projection.
 * K AllGather split 4 ways (one per head-pair tile) with explicit
   input-side dep edges (the barrier used to order them by accident).
 * emission order K -> Q -> V -> attention: all PSUM-evacuations ride the
   Scalar engine while it is otherwise idle (before the first Exp); the
   Act engine then runs Exp back-to-back with nothing else on its queue.
 * score matmuls emitted hh-alternated: consecutive MMs target row groups
   0-63 / 64-127 so the PE streams both heads' scores concurrently
   (row-group tiling) -- halves the score streaming cycles.
 * softmax rowsum reciprocals in two batches (heads 0-11 after t=5,
   heads 12-15 at the tail); heads 0-11 are normalized mid-attention so
   the out-proj can start immediately at exp-end (kd=6,7 deferred).
 * V bias folded into the residual on host (bv @ Wo term), bc broadcast
   read directly from PSUM by the DVE (no Scalar evac).
"""
import numpy as np
import ml_dtypes

import concourse.bass as bass
import concourse.mybir as mybir
import concourse.tile as tile
from concourse.bass_utils import run_bass_kernel_spmd

BF16 = ml_dtypes.bfloat16
bf16 = mybir.dt.bfloat16
f32 = mybir.dt.float32
AF = mybir.ActivationFunctionType
ALU = mybir.AluOpType
AX = mybir.AxisListType

B, S, DM = 2, 2048, 1024
H, D = 16, 64
NC = 8
ROWS = S * B // NC          # 512 query rows per core
SB = S


# ---- TileContext tail-drain patch: this walrus rejects >1 sync wait on
# CTRL-class instructions; split the global-clock waits onto SP nops.
def _patched_drain_and_barrier(self, tick_clock, wait_clock):
    nc = self.nc
    drain_inst = nc.sync.drain()
    wait_clock.add_sem_waits(
        drain_inst.ins, tile.ScopedClock({None: tick_clock.global_clock})
    )
    si = drain_inst.ins.sync_info
    waits = list(si.on_wait) if si and si.on_wait else []
    if len(waits) > 1:
        si.on_wait = waits[:1]
        for w in waits[1:]:
            nop = nc.sync.nop()
            nop.ins.sync_info = mybir.SyncInfo(on_wait=[w], on_update=[])
    nc.all_engine_barrier()
    assert self.sems is not None
    popped = nc._tile_sem_poison_stack.pop()
    assert popped is self._sem_poison
    nc.all_engine_barrier()


tile.TileContext._drain_and_barrier = _patched_drain_and_barrier

_CTRL_CLASSES = ("InstNoOp", "InstDrain", "InstEventSemaphore")


def _split_excess_waits(nc, maxw_compute=1):
    """Walrus (this version) caps sync waits per instruction (1 for
    CTRL-class, ~2 for compute).  Hoist excess waits onto same-engine NoOps
    inserted immediately before the offending instruction."""
    import copy
    proto = nc.sync.nop().ins  # prototype NoOp (appended to current bb; harmless)
    proto_si = proto.sync_info
    if proto_si and proto_si.on_wait:
        proto.sync_info = mybir.SyncInfo(on_wait=[], on_update=[])
    nsplit = 0
    for f in nc.m.functions:
        for b in f.blocks:
            insts = list(b.instructions)
            out = []
            changed = False
            for inst in insts:
                cls = type(inst).__name__
                maxw = 1 if cls in _CTRL_CLASSES else maxw_compute
                si = inst.sync_info
                waits = list(si.on_wait) if si and si.on_wait else []
                if len(waits) > maxw:
                    keep = waits[:maxw]
                    extra = waits[maxw:]
                    si.on_wait = keep
                    for i, w in enumerate(extra):
                        nop = copy.deepcopy(proto)
                        nop.name = f"{inst.name}-wsplit{i}"
                        nop.engine = inst.engine
                        nop.sync_info = mybir.SyncInfo(on_wait=[w],
                                                       on_update=[])
                        out.append(nop)
                        nsplit += 1
                    changed = True
                out.append(inst)
            if changed:
                try:
                    b.instructions = out
                except Exception:
                    b.set_instructions(out)
    return nsplit


def _build_program():
    nc = bass.Bass("TRN2", target_bir_lowering=False, debug=False,
                   num_devices=NC)

    def din(name, shape, dt):
        return nc.dram_tensor(name, list(shape), dt, kind="ExternalInput").ap()

    xT = din("xT", (DM, SB), bf16)
    xTq = din("xTq", (DM, ROWS), bf16)
    xr = din("xr", (ROWS, DM), f32)          # x rows + bo + bv@Wo (host)
    wq = din("wq", (DM, DM), bf16)
    wk = din("wk", (DM, DM), bf16)
    wv = din("wv", (DM, DM), bf16)
    wo = din("wo", (DM, DM), bf16)
    ccr = din("ccr", (128, ROWS), bf16)
    ssr = din("ssr", (128, ROWS), bf16)
    bqp = din("bqp", (128, 8), f32)
    bkp = din("bkp", (128, 8), f32)
    gbc = din("gbc", (128, DM), bf16)
    bbc = din("bbc", (128, DM), bf16)
    out = nc.dram_tensor("out", [ROWS, DM], f32, kind="ExternalOutput").ap()
    rs_dram = [nc.dram_tensor(f"rs_stage{g}", [1, (14 if g == 0 else 2) * 512],
                              bf16, kind="Internal").ap() for g in range(2)]
    rinv_dram = [nc.dram_tensor(f"rinv_stage{g}", [14 if g == 0 else 2, 512],
                                bf16, kind="Internal").ap() for g in range(2)]

    RG = [[0, 1, 2, 3], [4, 5, 6, 7]]

    with tile.TileContext(nc) as tc:
        with tc.tile_pool(name="res", bufs=1) as res, \
             tc.tile_pool(name="tmp", bufs=3) as tmp, \
             tc.tile_pool(name="ppool", bufs=5) as ppool, \
             tc.tile_pool(name="dram", bufs=1, space="DRAM") as dpool:

            xq_sb = [res.tile([128, ROWS], bf16, name=f"xq{k}", tag=f"xq{k}") for k in range(8)]
            kT = [res.tile([128, SB], bf16, name=f"kT{t}", tag=f"kT{t}") for t in range(8)]
            qT = [res.tile([128, ROWS], bf16, name=f"qT{t}", tag=f"qT{t}") for t in range(8)]
            vt = [res.tile([128, H * (D + 1)], bf16, name=f"vt{m}", tag=f"vt{m}")
                  for m in range(16)]
            aT = [res.tile([128, ROWS], bf16, name=f"aT{t}", tag=f"aT{t}") for t in range(8)]
            ccr_sb = res.tile([128, ROWS], bf16, tag="ccr")
            ssr_sb = res.tile([128, ROWS], bf16, tag="ssr")
            bq_sb = res.tile([128, 8], f32, tag="bq")
            bk_sb = res.tile([128, 8], f32, tag="bk")
            eps_sb = res.tile([128, 1], f32, tag="eps")

            # load order: K-proj inputs first (wk, xq, rope tables,
            # bias), then wq, wv, and the V-proj x^T tiles last (V MMs
            # start only after K+Q drain anyway).
            for k in range(8):
                nc.sync.dma_start(xq_sb[k][:], xTq[k * 128:(k + 1) * 128, :])
            nc.sync.dma_start(ccr_sb[:], ccr[:])
            nc.sync.dma_start(ssr_sb[:], ssr[:])
            nc.sync.dma_start(bk_sb[:], bkp[:])
            nc.sync.dma_start(bq_sb[:], bqp[:])
            nc.vector.memset(eps_sb[:], 1e-5)

            def rope(dst, src, cct, sst, n0, nn):
                # dst[:, n0:n0+nn] = src*CC + swap32(src)*SS
                # (cross-partition 2-input DVE ops are illegal -> copy first)
                t1 = tmp.tile([128, nn], bf16, tag="ropet1")
                t2 = tmp.tile([128, nn], bf16, tag="ropet2")
                for a, b_ in ((0, 32), (32, 0), (64, 96), (96, 64)):
                    nc.vector.tensor_copy(t2[a:a + 32, :], src[b_:b_ + 32, :])
                nc.vector.tensor_tensor(out=t1[:], in0=src[:],
                                        in1=cct[:, n0:n0 + nn], op=ALU.mult)
                nc.vector.tensor_tensor(out=t2[:], in0=t2[:],
                                        in1=sst[:, n0:n0 + nn], op=ALU.mult)
                nc.vector.tensor_tensor(out=dst[:, n0:n0 + nn], in0=t1[:],
                                        in1=t2[:], op=ALU.add)

            # ---- projections ----
            # Each core projects K only for its OWN 512 positions; four
            # 4-way AllGathers (one per head-pair tile pair, per batch
            # replica group) exchange the RoPEd K^T blocks while the PE
            # does Q and the (redundant) V projection.
            with tc.tile_pool(name="wts", bufs=1) as wts, \
                 tc.tile_pool(name="psP", bufs=4, space="PSUM") as psP:
                kin_q = [dpool.tile([256, 512], bf16, name=f"kin_{g}")
                         for g in range(4)]
                kout_q = [dpool.tile([1024, 512], bf16, name=f"kout_{g}")
                          for g in range(4)]

                wk_sb = [wts.tile([128, DM], bf16, name=f"wk{k}", tag=f"wk{k}")
                         for k in range(8)]
                wv_sb = [wts.tile([128, DM], bf16, name=f"wv{k}", tag=f"wv{k}")
                         for k in range(8)]
                xt_sb = [wts.tile([128, SB], bf16, name=f"xt{k}",
                                  tag=f"xt{k}") for k in range(8)]
                kT_own = [wts.tile([128, 512], bf16, name=f"ko{t}",
                                   tag=f"ko{t}") for t in range(8)]
                for k in range(8):
                    nc.sync.dma_start(wk_sb[k][:], wk[k * 128:(k + 1) * 128, :])

                # K^T projection (own 512 positions) + RoPE, then AllGather
                cc_k = [None] * 4
                kin_dmas = []
                for t in range(8):
                    ps = psP.tile([128, 512], f32, tag="proj")
                    for kd in range(8):
                        nc.tensor.matmul(
                            ps[:], wk_sb[kd][:, t * 128:(t + 1) * 128],
                            xq_sb[kd][:], start=(kd == 0), stop=(kd == 7))
                    kt_raw = tmp.tile([128, 512], bf16, tag="evac")
                    nc.scalar.activation(kt_raw[:], ps[:], AF.Identity,
                                         bias=bk_sb[:, t:t + 1])
                    rope(kT_own[t], kt_raw, ccr_sb, ssr_sb, 0, 512)
                    g, tt = t // 2, t % 2
                    # staged from the scalar queue: a sync-queue DMA here
                    # would head-of-line-block the wq/xT/wv loads behind it
                    # while waiting on the RoPE.
                    dma = nc.scalar.dma_start(
                        kin_q[g][tt * 128:(tt + 1) * 128, :], kT_own[t][:])
                    kin_dmas.append(dma)
                    if tt == 1:
                        cc = nc.gpsimd.collective_compute(
                            "AllGather", ALU.bypass, replica_groups=RG,
                            ins=[kin_q[g].opt()], outs=[kout_q[g].opt()])
                        # DRAM tiles are not dependency-tracked: tie the
                        # trigger to the two staging DMAs explicitly.
                        for d_ in kin_dmas[-2:]:
                            bass._add_dep_helper(cc.ins, d_.ins, sync=True,
                                                 reason="AG_K input staged")
                        cc_k[g] = cc

                # Q^T projection + RoPE (wq reuses wk slots)
                wq_sb = [wts.tile([128, DM], bf16, name=f"wq{k}", tag=f"wk{k}")
                         for k in range(8)]
                for k in range(8):
                    nc.sync.dma_start(wq_sb[k][:], wq[k * 128:(k + 1) * 128, :])
                # wv + x^T stream right behind wq (before the Q-proj
                # emission) so the sync queue never idles the HBM port.
                for k in range(8):
                    nc.sync.dma_start(wv_sb[k][:], wv[k * 128:(k + 1) * 128, :])
                for k in range(8):
                    nc.sync.dma_start(xt_sb[k][:], xT[k * 128:(k + 1) * 128, :])
                for t in range(8):
                    ps = psP.tile([128, 512], f32, tag="proj")
                    for kd in range(8):
                        nc.tensor.matmul(
                            ps[:], wq_sb[kd][:, t * 128:(t + 1) * 128],
                            xq_sb[kd][:], start=(kd == 0), stop=(kd == 7))
                    q_raw = tmp.tile([128, ROWS], bf16, tag="evac")
                    nc.scalar.activation(q_raw[:], ps[:], AF.Identity,
                                         bias=bq_sb[:, t:t + 1])
                    rope(qT[t], q_raw, ccr_sb, ssr_sb, 0, ROWS)

                # V projection (redundant, all 2048 positions; 65-stride
                # head slots + ones column for the softmax rowsums).
                # Scalar-engine evac: Act is idle until the first Exp.
                for m in range(16):
                    m0 = m * 128
                    for ncol in range(2):
                        c0 = ncol * 512
                        ps = psP.tile([128, 512], f32, tag="proj")
                        for kd in range(8):
                            nc.tensor.matmul(
                                ps[:], xt_sb[kd][:, m0:m0 + 128],
                                wv_sb[kd][:, c0:c0 + 512],
                                start=(kd == 0), stop=(kd == 7))
                        dst = vt[m][:, ncol * 8 * 65:(ncol + 1) * 8 * 65]
                        dstv = dst.rearrange("p (h e) -> p h e", e=65)[:, :, 0:64]
                        srcv = ps[:].rearrange("p (h e) -> p h e", e=64)
                        nc.scalar.activation(dstv, srcv, AF.Identity)
                    onev = vt[m][:, :].rearrange("p (h e) -> p h e",
                                                 e=65)[:, :, 64:65]
                    nc.vector.memset(onev, 1.0)

                # gathered K^T -> attention layout (sync queue, after all
                # critical loads; explicit dep edges onto the collectives).
                for g in range(4):
                    for i in range(4):
                        for tt in range(2):
                            t = g * 2 + tt
                            dma = nc.sync.dma_start(
                                kT[t][:, i * 512:(i + 1) * 512],
                                kout_q[g][i * 256 + tt * 128:
                                          i * 256 + (tt + 1) * 128, :])
                            bass._add_dep_helper(dma.ins, cc_k[g].ins,
                                                 sync=True,
                                                 reason="AG_K output read")

            # ---- attention ----
            # scores transposed (S^T = K^T-chunk @ Q^T) into [128,1024] PSUM
            # mega-tiles so each Exp covers FD=1024.  The two heads of a
            # t-pair are emitted alternating so their score MMs stream
            # concurrently through PE row groups 0-63 / 64-127.  PV lags one
            # kc-pair.  Rowsums ride the V ones-column; normalization is
            # deferred and applied to the bf16 aT tiles -- heads 0-11
            # mid-attention (after t=5), heads 12-15 at the tail.
            with tc.tile_pool(name="asb", bufs=1) as asb:
                # rowsum rows stage into one SBUF row; a DRAM bounce
                # scatters them across partitions so ONE batched DVE
                # reciprocal runs at FD=512 (DVE reciprocal is iterative,
                # ~6.5ns/elem along the free dim -- partition-packing is
                # what makes it cheap).  rinvA holds the scatter back at
                # partitions {0,32}: head 2g+i -> partition 32i, col g*512.
                rs_row = asb.tile([65, H * 512], bf16, tag="rs_row")
                rsS = [asb.tile([14 if g == 0 else 2, 512], bf16,
                                tag=f"rsS{g}", name=f"rsS{g}")
                       for g in range(2)]
                rinvS = [asb.tile([14 if g == 0 else 2, 512], bf16,
                                  tag=f"rinvS{g}", name=f"rinvS{g}")
                         for g in range(2)]
                rinvA = asb.tile([33, 8 * 512], bf16, tag="rinvA")
                onesA = asb.tile([33, 64], bf16, tag="onesA")
                nc.vector.memset(onesA[:], 1.0)

                with tc.tile_pool(name="psA", bufs=3, space="PSUM") as psA, \
                     tc.tile_pool(name="psO", bufs=2, space="PSUM") as psO:

                    for t in range(8):
                        oaccs = [psO.tile([65, 512], f32, tag="oacc",
                                          name=f"oacc{t}_{hh}")
                                 for hh in range(2)]
                        prev = [None, None]

                        def emit_pv(hh, kp, pT_t):
                            h = 2 * t + hh
                            for j in range(2):
                                kc = kp * 2 + j
                                nc.tensor.matmul(
                                    oaccs[hh][:],
                                    vt[kc][:, h * 65:h * 65 + 65],
                                    pT_t[:, j * 512:(j + 1) * 512],
                                    start=(kc == 0), stop=(kc == 15))

                        for kp in range(8):
                            sps = [psA.tile([128, 1024], f32, tag="sco",
                                            name=f"sco{t}_{kp}_{hh}")
                                   for hh in range(2)]
                            # hh-alternated score MMs: row groups 0-63 and
                            # 64-127 stream concurrently.
                            for j in range(2):
                                kc = kp * 2 + j
                                for hh in range(2):
                                    po = 64 * hh
                                    nc.tensor.matmul(
                                        sps[hh][:, j * 512:(j + 1) * 512],
                                        kT[t][po:po + 64,
                                              kc * 128:(kc + 1) * 128],
                                        qT[t][po:po + 64, :],
                                        start=True, stop=True)
                            for hh in range(2):
                                pT = ppool.tile([128, 1024], bf16, tag="pT")
                                nc.scalar.activation(pT[:], sps[hh][:],
                                                     AF.Exp, scale=0.125)
                                if prev[hh] is not None:
                                    emit_pv(hh, kp - 1, prev[hh])
                                prev[hh] = pT
                        for hh in range(2):
                            emit_pv(hh, 7, prev[hh])
                        # stash rowsum rows (same-partition copies) +
                        # unnorm. attn (out-partition shift legal for
                        # 1-input copies)
                        for hh in range(2):
                            h, po = 2 * t + hh, 64 * hh
                            nc.vector.tensor_copy(
                                rs_row[64:65, h * 512:(h + 1) * 512],
                                oaccs[hh][64:65, :])
                            nc.vector.tensor_copy(aT[t][po:po + 64, :],
                                                  oaccs[hh][0:64, :])
                        # reciprocal batches: heads 0-13 after t=6 (the
                        # bounce + recip hide inside t=7's exp window),
                        # heads 14-15 at the tail.
                        if t in (6, 7):
                            gb = 0 if t == 6 else 1
                            nh = 14 if t == 6 else 2
                            c0 = 0 if t == 6 else 14 * 512
                            nc.sync.dma_start(rs_dram[gb][:],
                                              rs_row[64:65, c0:c0 + nh * 512])
                            nc.sync.dma_start(
                                rsS[gb][:],
                                rs_dram[gb].rearrange("a (p c) -> (a p) c",
                                                      p=nh))
                            with nc.allow_low_precision(
                                    reason="softmax 1/rowsum in bf16"):
                                nc.vector.reciprocal(rinvS[gb][:], rsS[gb][:])
                            nc.sync.dma_start(rinv_dram[gb][:], rinvS[gb][:])
                            for i in range(2):
                                nc.sync.dma_start(
                                    rinvA[32 * i:32 * i + 1,
                                          (c0 // 2):(c0 // 2) + (nh // 2) * 512
                                          ].rearrange("a (g c) -> a g c",
                                                      c=512),
                                    rinv_dram[gb].rearrange(
                                        "(g i) c -> i g c", i=2)[i:i + 1])
                # ---- out-proj + residual + LayerNorm ----
                # (psA/psO closed -> PSUM free for psF + psB.)
                # All 16 head-normalizes run here, interleaved with the
                # out-proj kd accumulation so the PE never waits: head
                # pair 2t,2t+1 normalizes, then kd=t accumulates.
                with tc.tile_pool(name="wop", bufs=1) as wop, \
                     tc.tile_pool(name="fin", bufs=2) as fin, \
                     tc.tile_pool(name="psB", bufs=2, space="PSUM") as psB, \
                     tc.tile_pool(name="psF", bufs=4, space="PSUM") as psF:

                    def normalize_pair(t):
                        # both heads' 1/rowsum broadcasts via col-tiled PE
                        # outer products into ONE PSUM tile; Act (idle
                        # post-Exp, Identity stays in the exp table set)
                        # evacuates so the DVE multiply runs in 2x bf16
                        # mode as a single [128,512] op.
                        bc = psB.tile([128, 512], f32, tag="bc")
                        for hh in range(2):
                            po = 64 * hh
                            nc.tensor.matmul(
                                bc[po:po + 64, :],
                                onesA[32 * hh:32 * hh + 1, :],
                                rinvA[32 * hh:32 * hh + 1,
                                      t * 512:(t + 1) * 512],
                                start=True, stop=True)
                        bcs = tmp.tile([128, 512], bf16, tag="bcs")
                        nc.scalar.activation(bcs[:], bc[:], AF.Identity)
                        nc.vector.tensor_tensor(out=aT[t][:], in0=aT[t][:],
                                                in1=bcs[:], op=ALU.mult)
                    wo_sb = [wop.tile([128, DM], bf16, name=f"wo{k}",
                                      tag=f"wo{k}") for k in range(8)]
                    for k in range(8):
                        nc.sync.dma_start(wo_sb[k][:],
                                          wo[k * 128:(k + 1) * 128, :])
                    g_sb = wop.tile([128, DM], bf16, tag="g")
                    b_sb = wop.tile([128, DM], bf16, tag="b")
                    nc.sync.dma_start(g_sb[:], gbc[:])
                    nc.sync.dma_start(b_sb[:], bbc[:])

                    def outproj(psf, mrs, kds):
                        for kd in kds:
                            for mr in mrs:
                                rr = mr * 128
                                for ncol in range(2):
                                    c0 = ncol * 512
                                    nc.tensor.matmul(
                                        psf[(mr, ncol)][:],
                                        aT[kd][:, rr:rr + 128],
                                        wo_sb[kd][:, c0:c0 + 512],
                                        start=(kd == 0), stop=(kd == 7))

                    def ln_rows(psf, mr):
                        rr = mr * 128
                        xb = fin.tile([128, DM], f32, tag="xb")
                        nc.sync.dma_start(xb[:], xr[rr:rr + 128, :])
                        # LN arithmetic in bf16: 2x/4x DVE modes; abs error
                        # ~2e-3 of a ~5.0-max output, well inside tolerance
                        hrow = fin.tile([128, DM], bf16, tag="hrow")
                        for ncol in range(2):
                            c0 = ncol * 512
                            nc.vector.tensor_tensor(
                                out=hrow[:, c0:c0 + 512],
                                in0=psf[(mr, ncol)][:],
                                in1=xb[:, c0:c0 + 512], op=ALU.add)
                        ssum = fin.tile([128, 1], f32, tag="ssum")
                        nc.vector.reduce_sum(out=ssum[:], in_=hrow[:],
                                             axis=AX.X)
                        mu = fin.tile([128, 1], f32, tag="mu")
                        nc.vector.tensor_scalar(out=mu[:], in0=ssum[:],
                                                scalar1=1.0 / DM,
                                                scalar2=None, op0=ALU.mult)
                        d = fin.tile([128, DM], bf16, tag="d")
                        nc.vector.tensor_scalar(out=d[:], in0=hrow[:],
                                                scalar1=mu[:], scalar2=None,
                                                op0=ALU.subtract)
                        y = fin.tile([128, DM], bf16, tag="y")
                        vs = fin.tile([128, 1], f32, tag="vs")
                        nc.vector.tensor_tensor(out=y[:], in0=d[:], in1=d[:],
                                                op=ALU.mult)
                        nc.vector.reduce_sum(out=vs[:], in_=y[:], axis=AX.X)
                        st = fin.tile([128, 1], f32, tag="st")
                        nc.scalar.activation(st[:], vs[:], AF.Sqrt,
                                             bias=eps_sb[:], scale=1.0 / DM)
                        rstd = fin.tile([128, 1], f32, tag="rstd")
                        nc.vector.reciprocal(rstd[:], st[:])
                        nc.vector.tensor_scalar(out=y[:], in0=d[:],
                                                scalar1=rstd[:],
                                                scalar2=None, op0=ALU.mult)
                        nc.vector.tensor_tensor(out=y[:], in0=y[:],
                                                in1=g_sb[:], op=ALU.mult)
                        yf = fin.tile([128, DM], f32, tag="yf")
                        nc.vector.tensor_tensor(out=yf[:], in0=y[:],
                                                in1=b_sb[:], op=ALU.add)
                        nc.sync.dma_start(out[rr:rr + 128, :], yf[:])

                    # 4 psF tags x 1 buf = 4 banks; half 2 reuses half 1's
                    # slots (waits on the half-1 LN PSUM reads).
                    psf1 = {(mr, ncol): psF.tile([128, 512], f32, bufs=1,
                                                 tag=f"f{mr}_{ncol}",
                                                 name=f"psfa{mr}_{ncol}")
                            for mr in (0, 1) for ncol in (0, 1)}
                    for t in range(8):
                        normalize_pair(t)
                        outproj(psf1, (0, 1), (t,))
                    for mr in (0, 1):
                        ln_rows(psf1, mr)
                    # emitted after the half-1 LN reads so the slot-reuse
                    # anti-dependency edges are recorded.
                    psf2 = {(mr, ncol): psF.tile([128, 512], f32, bufs=1,
                                                 tag=f"f{mr - 2}_{ncol}",
                                                 name=f"psfb{mr}_{ncol}")
                            for mr in (2, 3) for ncol in (0, 1)}
                    outproj(psf2, (2, 3), range(8))
                    for mr in (2, 3):
                        ln_rows(psf2, mr)

    _split_excess_waits(nc)
    return nc


_NC_CACHE = None


def _perm():
    p = np.zeros(DM, np.int64)
    for h in range(H):
        p[h * D:h * D + 32] = h * D + np.arange(0, D, 2)
        p[h * D + 32:(h + 1) * D] = h * D + np.arange(1, D, 2)
    return p


def kernel(x, Wqkv, bqkv, Wo, bo, gamma, beta):
    global _NC_CACHE
    x = np.asarray(x, np.float32)
    Wqkv = np.asarray(Wqkv, np.float32)
    bqkv = np.asarray(bqkv, np.float32)
    Wo = np.asarray(Wo, np.float32)
    bo = np.asarray(bo, np.float32)
    gamma = np.asarray(gamma, np.float32)
    beta = np.asarray(beta, np.float32)

    perm = _perm()
    Wq = np.ascontiguousarray(Wqkv[:, 0:DM][:, perm]).astype(BF16)
    Wk = np.ascontiguousarray(Wqkv[:, DM:2 * DM][:, perm]).astype(BF16)
    Wv = np.ascontiguousarray(Wqkv[:, 2 * DM:3 * DM]).astype(BF16)
    Wob = Wo.astype(BF16)
    bq = bqkv[0:DM][perm]
    bk = bqkv[DM:2 * DM][perm]
    bv = bqkv[2 * DM:3 * DM]

    inv = 1.0 / (10000.0 ** (np.arange(0, D, 2, dtype=np.float64) / D))
    pos = np.arange(S, dtype=np.float64)
    fr = pos[None, :] * inv[:, None]                    # [32, S]
    c32, s32 = np.cos(fr), np.sin(fr)
    CC = np.concatenate([c32, c32, c32, c32], 0).astype(BF16)   # [128, S]
    SS = np.concatenate([-s32, s32, -s32, s32], 0).astype(BF16)

    def colmajor(v):
        return np.ascontiguousarray(v.reshape(8, 128).T).astype(np.float32)

    gB = np.ascontiguousarray(np.broadcast_to(gamma, (128, DM))).astype(BF16)
    bB = np.ascontiguousarray(np.broadcast_to(beta, (128, DM))).astype(BF16)
    # bv contributes bv @ Wo to every output row (softmax weights sum to 1);
    # fold it, with bo, into the residual rows on host.
    rbias = bo + bv @ Wo

    if _NC_CACHE is None:
        _NC_CACHE = _build_program()
    nc = _NC_CACHE

    in_maps = []
    for c in range(NC):
        b, r = c // 4, c % 4
        xTb = np.ascontiguousarray(x[b].T).astype(BF16)
        rr = r * ROWS
        in_maps.append({
            "xT": xTb,
            "xTq": np.ascontiguousarray(xTb[:, rr:rr + ROWS]),
            "xr": np.ascontiguousarray(x[b, rr:rr + ROWS, :] + rbias[None, :]),
            "wq": Wq, "wk": Wk, "wv": Wv, "wo": Wob,
            "ccr": np.ascontiguousarray(CC[:, rr:rr + ROWS]),
            "ssr": np.ascontiguousarray(SS[:, rr:rr + ROWS]),
            "bqp": colmajor(bq), "bkp": colmajor(bk),
            "gbc": gB, "bbc": bB,
        })

    res = run_bass_kernel_spmd(nc, in_maps, core_ids=list(range(NC)))
    kernel._last_results = res
    full = np.empty((B, S, DM), np.float32)
    for c in range(NC):
        b, r = c // 4, c % 4
        full[b, r * ROWS:(r + 1) * ROWS, :] = res.results[c]["out"]
    return full


# revision 26
# speedup vs baseline: 1.3384x; 1.0882x over previous
"""Trainium2 Bass kernel for nn_LocalSelfAttention (fused attention block).

Reference (B=2, S=2048, DM=1024, H=16, D=64):
  qkv = x @ Wqkv + bqkv -> split heads -> RoPE(q,k) -> softmax(q k^T/8) v
  -> concat heads @ Wo + bo -> residual + LayerNorm(gamma,beta)

Sharding (8 cores): core c = (batch c//4, query rows 512*(c%4)..+512).
K^T is projected per-core for its OWN 512 positions only and exchanged by
4-way AllGathers per batch replica group; V is recomputed redundantly.
Attention/out-proj/LN are exact and row-local; host gather is pure
concatenation.

v2 changes vs baseline (363.8us):
 * dummy 256B collective issued first: absorbs the ~45us first-collective
   entry barrier under the input DMA loads + K projection.
 * K AllGather split 4 ways (one per head-pair tile) with explicit
   input-side dep edges (the barrier used to order them by accident).
 * emission order K -> Q -> V -> attention: all PSUM-evacuations ride the
   Scalar engine while it is otherwise idle (before the first Exp); the
   Act engine then runs Exp back-to-back with nothing else on its queue.
 * score matmuls emitted hh-alternated: consecutive MMs target row groups
   0-63 / 64-127 so the PE streams both heads' scores concurrently
   (row-group tiling) -- halves the score streaming cycles.
 * softmax rowsum reciprocals in two batches (heads 0-11 after t=5,
   heads 12-15 at the tail); heads 0-11 are normalized mid-attention so
   the out-proj can start immediately at exp-end (kd=6,7 deferred).
 * V bias folded into the residual on host (bv @ Wo term), bc broadcast
   read directly from PSUM by the DVE (no Scalar evac).
"""
import numpy as np
import ml_dtypes

import concourse.bass as bass
import concourse.mybir as mybir
import concourse.tile as tile
from concourse.bass_utils import run_bass_kernel_spmd

BF16 = ml_dtypes.bfloat16
bf16 = mybir.dt.bfloat16
f32 = mybir.dt.float32
AF = mybir.ActivationFunctionType
ALU = mybir.AluOpType
AX = mybir.AxisListType

B, S, DM = 2, 2048, 1024
H, D = 16, 64
NC = 8
ROWS = S * B // NC          # 512 query rows per core
SB = S


# ---- TileContext tail-drain patch: this walrus rejects >1 sync wait on
# CTRL-class instructions; split the global-clock waits onto SP nops.
def _patched_drain_and_barrier(self, tick_clock, wait_clock):
    nc = self.nc
    drain_inst = nc.sync.drain()
    wait_clock.add_sem_waits(
        drain_inst.ins, tile.ScopedClock({None: tick_clock.global_clock})
    )
    si = drain_inst.ins.sync_info
    waits = list(si.on_wait) if si and si.on_wait else []
    if len(waits) > 1:
        si.on_wait = waits[:1]
        for w in waits[1:]:
            nop = nc.sync.nop()
            nop.ins.sync_info = mybir.SyncInfo(on_wait=[w], on_update=[])
    nc.all_engine_barrier()
    assert self.sems is not None
    popped = nc._tile_sem_poison_stack.pop()
    assert popped is self._sem_poison
    nc.all_engine_barrier()


tile.TileContext._drain_and_barrier = _patched_drain_and_barrier

_CTRL_CLASSES = ("InstNoOp", "InstDrain", "InstEventSemaphore")


def _split_excess_waits(nc, maxw_compute=1):
    """Walrus (this version) caps sync waits per instruction (1 for
    CTRL-class, ~2 for compute).  Hoist excess waits onto same-engine NoOps
    inserted immediately before the offending instruction."""
    import copy
    proto = nc.sync.nop().ins  # prototype NoOp (appended to current bb; harmless)
    proto_si = proto.sync_info
    if proto_si and proto_si.on_wait:
        proto.sync_info = mybir.SyncInfo(on_wait=[], on_update=[])
    nsplit = 0
    for f in nc.m.functions:
        for b in f.blocks:
            insts = list(b.instructions)
            out = []
            changed = False
            for inst in insts:
                cls = type(inst).__name__
                maxw = 1 if cls in _CTRL_CLASSES else maxw_compute
                si = inst.sync_info
                waits = list(si.on_wait) if si and si.on_wait else []
                if len(waits) > maxw:
                    keep = waits[:maxw]
                    extra = waits[maxw:]
                    si.on_wait = keep
                    for i, w in enumerate(extra):
                        nop = copy.deepcopy(proto)
                        nop.name = f"{inst.name}-wsplit{i}"
                        nop.engine = inst.engine
                        nop.sync_info = mybir.SyncInfo(on_wait=[w],
                                                       on_update=[])
                        out.append(nop)
                        nsplit += 1
                    changed = True
                out.append(inst)
            if changed:
                try:
                    b.instructions = out
                except Exception:
                    b.set_instructions(out)
    return nsplit


def _build_program():
    nc = bass.Bass("TRN2", target_bir_lowering=False, debug=False,
                   num_devices=NC)

    def din(name, shape, dt):
        return nc.dram_tensor(name, list(shape), dt, kind="ExternalInput").ap()

    xT = din("xT", (DM, SB), bf16)
    xTq = din("xTq", (DM, ROWS), bf16)
    xr = din("xr", (ROWS, DM), f32)          # x rows + bo + bv@Wo (host)
    wq = din("wq", (DM, DM), bf16)
    wk = din("wk", (DM, DM), bf16)
    wv = din("wv", (DM, DM), bf16)
    wo = din("wo", (DM, DM), bf16)
    ccr = din("ccr", (128, ROWS), bf16)
    ssr = din("ssr", (128, ROWS), bf16)
    bqp = din("bqp", (128, 8), f32)
    bkp = din("bkp", (128, 8), f32)
    gbc = din("gbc", (128, DM), bf16)
    bbc = din("bbc", (128, DM), bf16)
    out = nc.dram_tensor("out", [ROWS, DM], f32, kind="ExternalOutput").ap()
    rs_dram = [nc.dram_tensor(f"rs_stage{g}", [1, (14 if g == 0 else 2) * 512],
                              bf16, kind="Internal").ap() for g in range(2)]
    rinv_dram = [nc.dram_tensor(f"rinv_stage{g}", [14 if g == 0 else 2, 512],
                                bf16, kind="Internal").ap() for g in range(2)]

    RG = [[0, 1, 2, 3], [4, 5, 6, 7]]

    with tile.TileContext(nc) as tc:
        with tc.tile_pool(name="res", bufs=1) as res, \
             tc.tile_pool(name="tmp", bufs=3) as tmp, \
             tc.tile_pool(name="ppool", bufs=5) as ppool, \
             tc.tile_pool(name="dram", bufs=1, space="DRAM") as dpool:

            xq_sb = [res.tile([128, ROWS], bf16, name=f"xq{k}", tag=f"xq{k}") for k in range(8)]
            kT = [res.tile([128, SB], bf16, name=f"kT{t}", tag=f"kT{t}") for t in range(8)]
            qT = [res.tile([128, ROWS], bf16, name=f"qT{t}", tag=f"qT{t}") for t in range(8)]
            vt = [res.tile([128, H * (D + 1)], bf16, name=f"vt{m}", tag=f"vt{m}")
                  for m in range(16)]
            aT = [res.tile([128, ROWS], bf16, name=f"aT{t}", tag=f"aT{t}") for t in range(8)]
            ccr_sb = res.tile([128, ROWS], bf16, tag="ccr")
            ssr_sb = res.tile([128, ROWS], bf16, tag="ssr")
            bq_sb = res.tile([128, 8], f32, tag="bq")
            bk_sb = res.tile([128, 8], f32, tag="bk")
            eps_sb = res.tile([128, 1], f32, tag="eps")

            # load order: K-proj inputs first (wk, xq, rope tables,
            # bias), then wq, wv, and the V-proj x^T tiles last (V MMs
            # start only after K+Q drain anyway).
            for k in range(8):
                nc.sync.dma_start(xq_sb[k][:], xTq[k * 128:(k + 1) * 128, :])
            nc.sync.dma_start(ccr_sb[:], ccr[:])
            nc.sync.dma_start(ssr_sb[:], ssr[:])
            nc.sync.dma_start(bk_sb[:], bkp[:])
            nc.sync.dma_start(bq_sb[:], bqp[:])
            nc.vector.memset(eps_sb[:], 1e-5)

            def rope(dst, src, cct, sst, n0, nn):
                # dst[:, n0:n0+nn] = src*CC + swap32(src)*SS
                # (cross-partition 2-input DVE ops are illegal -> copy first)
                t1 = tmp.tile([128, nn], bf16, tag="ropet1")
                t2 = tmp.tile([128, nn], bf16, tag="ropet2")
                for a, b_ in ((0, 32), (32, 0), (64, 96), (96, 64)):
                    nc.vector.tensor_copy(t2[a:a + 32, :], src[b_:b_ + 32, :])
                nc.vector.tensor_tensor(out=t1[:], in0=src[:],
                                        in1=cct[:, n0:n0 + nn], op=ALU.mult)
                nc.vector.tensor_tensor(out=t2[:], in0=t2[:],
                                        in1=sst[:, n0:n0 + nn], op=ALU.mult)
                nc.vector.tensor_tensor(out=dst[:, n0:n0 + nn], in0=t1[:],
                                        in1=t2[:], op=ALU.add)

            # ---- projections ----
            # Each core projects K only for its OWN 512 positions; four
            # 4-way AllGathers (one per head-pair tile pair, per batch
            # replica group) exchange the RoPEd K^T blocks while the PE
            # does Q and the (redundant) V projection.
            with tc.tile_pool(name="wts", bufs=1) as wts, \
                 tc.tile_pool(name="psP", bufs=4, space="PSUM") as psP:
                kin_q = [dpool.tile([256, 512], bf16, name=f"kin_{g}")
                         for g in range(4)]
                kout_q = [dpool.tile([1024, 512], bf16, name=f"kout_{g}")
                          for g in range(4)]

                wk_sb = [wts.tile([128, DM], bf16, name=f"wk{k}", tag=f"wk{k}")
                         for k in range(8)]
                wv_sb = [wts.tile([128, DM], bf16, name=f"wv{k}", tag=f"wv{k}")
                         for k in range(8)]
                xt_sb = [wts.tile([128, SB], bf16, name=f"xt{k}",
                                  tag=f"xt{k}") for k in range(8)]
                kT_own = [wts.tile([128, 512], bf16, name=f"ko{t}",
                                   tag=f"ko{t}") for t in range(8)]
                for k in range(8):
                    nc.sync.dma_start(wk_sb[k][:], wk[k * 128:(k + 1) * 128, :])

                # K^T projection (own 512 positions) + RoPE, then AllGather
                cc_k = [None] * 4
                kin_dmas = []
                for t in range(8):
                    ps = psP.tile([128, 512], f32, tag="proj")
                    for kd in range(8):
                        nc.tensor.matmul(
                            ps[:], wk_sb[kd][:, t * 128:(t + 1) * 128],
                            xq_sb[kd][:], start=(kd == 0), stop=(kd == 7))
                    kt_raw = tmp.tile([128, 512], bf16, tag="evac")
                    nc.scalar.activation(kt_raw[:], ps[:], AF.Identity,
                                         bias=bk_sb[:, t:t + 1])
                    rope(kT_own[t], kt_raw, ccr_sb, ssr_sb, 0, 512)
                    g, tt = t // 2, t % 2
                    # staged from the scalar queue: a sync-queue DMA here
                    # would head-of-line-block the wq/xT/wv loads behind it
                    # while waiting on the RoPE.
                    dma = nc.scalar.dma_start(
                        kin_q[g][tt * 128:(tt + 1) * 128, :], kT_own[t][:])
                    kin_dmas.append(dma)
                    if tt == 1:
                        cc = nc.gpsimd.collective_compute(
                            "AllGather", ALU.bypass, replica_groups=RG,
                            ins=[kin_q[g].opt()], outs=[kout_q[g].opt()])
                        # DRAM tiles are not dependency-tracked: tie the
                        # trigger to the two staging DMAs explicitly.
                        for d_ in kin_dmas[-2:]:
                            bass._add_dep_helper(cc.ins, d_.ins, sync=True,
                                                 reason="AG_K input staged")
                        cc_k[g] = cc

                # Q^T projection + RoPE (wq reuses wk slots)
                wq_sb = [wts.tile([128, DM], bf16, name=f"wq{k}", tag=f"wk{k}")
                         for k in range(8)]
                for k in range(8):
                    nc.sync.dma_start(wq_sb[k][:], wq[k * 128:(k + 1) * 128, :])
                # wv + x^T stream right behind wq (before the Q-proj
                # emission) so the sync queue never idles the HBM port.
                for k in range(8):
                    nc.sync.dma_start(wv_sb[k][:], wv[k * 128:(k + 1) * 128, :])
                for k in range(8):
                    nc.sync.dma_start(xt_sb[k][:], xT[k * 128:(k + 1) * 128, :])
                for t in range(8):
                    ps = psP.tile([128, 512], f32, tag="proj")
                    for kd in range(8):
                        nc.tensor.matmul(
                            ps[:], wq_sb[kd][:, t * 128:(t + 1) * 128],
                            xq_sb[kd][:], start=(kd == 0), stop=(kd == 7))
                    q_raw = tmp.tile([128, ROWS], bf16, tag="evac")
                    nc.scalar.activation(q_raw[:], ps[:], AF.Identity,
                                         bias=bq_sb[:, t:t + 1])
                    rope(qT[t], q_raw, ccr_sb, ssr_sb, 0, ROWS)

                # V projection (redundant, all 2048 positions; 65-stride
                # head slots + ones column for the softmax rowsums).
                # Scalar-engine evac: Act is idle until the first Exp.
                for m in range(16):
                    m0 = m * 128
                    pss = [psP.tile([128, 512], f32, tag="proj",
                                    name=f"vps{m}_{ncol}")
                           for ncol in range(2)]
                    # kd-inner: consecutive MM pairs share the stationary
                    # xt slice, so the weight load amortizes over 1024
                    # streamed columns.
                    for kd in range(8):
                        for ncol in range(2):
                            nc.tensor.matmul(
                                pss[ncol][:], xt_sb[kd][:, m0:m0 + 128],
                                wv_sb[kd][:, ncol * 512:ncol * 512 + 512],
                                start=(kd == 0), stop=(kd == 7))
                    for ncol in range(2):
                        dst = vt[m][:, ncol * 8 * 65:(ncol + 1) * 8 * 65]
                        dstv = dst.rearrange("p (h e) -> p h e", e=65)[:, :, 0:64]
                        srcv = pss[ncol][:].rearrange("p (h e) -> p h e", e=64)
                        nc.scalar.activation(dstv, srcv, AF.Identity)
                    onev = vt[m][:, :].rearrange("p (h e) -> p h e",
                                                 e=65)[:, :, 64:65]
                    nc.vector.memset(onev, 1.0)

                # gathered K^T -> attention layout (sync queue, after all
                # critical loads; explicit dep edges onto the collectives).
                for g in range(4):
                    for i in range(4):
                        for tt in range(2):
                            t = g * 2 + tt
                            dma = nc.sync.dma_start(
                                kT[t][:, i * 512:(i + 1) * 512],
                                kout_q[g][i * 256 + tt * 128:
                                          i * 256 + (tt + 1) * 128, :])
                            bass._add_dep_helper(dma.ins, cc_k[g].ins,
                                                 sync=True,
                                                 reason="AG_K output read")

            # ---- attention ----
            # scores transposed (S^T = K^T-chunk @ Q^T) into [128,1024] PSUM
            # mega-tiles so each Exp covers FD=1024.  The two heads of a
            # t-pair are emitted alternating so their score MMs stream
            # concurrently through PE row groups 0-63 / 64-127.  PV lags one
            # kc-pair.  Rowsums ride the V ones-column; normalization is
            # deferred and applied to the bf16 aT tiles -- heads 0-11
            # mid-attention (after t=5), heads 12-15 at the tail.
            with tc.tile_pool(name="asb", bufs=1) as asb:
                # rowsum rows stage into one SBUF row; a DRAM bounce
                # scatters them across partitions so ONE batched DVE
                # reciprocal runs at FD=512 (DVE reciprocal is iterative,
                # ~6.5ns/elem along the free dim -- partition-packing is
                # what makes it cheap).  rinvA holds the scatter back at
                # partitions {0,32}: head 2g+i -> partition 32i, col g*512.
                rs_row = asb.tile([65, H * 512], bf16, tag="rs_row")
                rsS = [asb.tile([14 if g == 0 else 2, 512], bf16,
                                tag=f"rsS{g}", name=f"rsS{g}")
                       for g in range(2)]
                rinvS = [asb.tile([14 if g == 0 else 2, 512], bf16,
                                  tag=f"rinvS{g}", name=f"rinvS{g}")
                         for g in range(2)]
                rinvA = asb.tile([33, 8 * 512], bf16, tag="rinvA")
                onesA = asb.tile([33, 64], bf16, tag="onesA")
                nc.vector.memset(onesA[:], 1.0)

                with tc.tile_pool(name="psA", bufs=3, space="PSUM") as psA, \
                     tc.tile_pool(name="psO", bufs=2, space="PSUM") as psO:

                    for t in range(8):
                        oaccs = [psO.tile([65, 512], f32, tag="oacc",
                                          name=f"oacc{t}_{hh}")
                                 for hh in range(2)]
                        prev = [None, None]

                        def emit_pv(hh, kp, pT_t):
                            h = 2 * t + hh
                            for j in range(2):
                                kc = kp * 2 + j
                                nc.tensor.matmul(
                                    oaccs[hh][:],
                                    vt[kc][:, h * 65:h * 65 + 65],
                                    pT_t[:, j * 512:(j + 1) * 512],
                                    start=(kc == 0), stop=(kc == 15))

                        for kp in range(8):
                            sps = [psA.tile([128, 1024], f32, tag="sco",
                                            name=f"sco{t}_{kp}_{hh}")
                                   for hh in range(2)]
                            # hh-alternated score MMs: row groups 0-63 and
                            # 64-127 stream concurrently.
                            for j in range(2):
                                kc = kp * 2 + j
                                for hh in range(2):
                                    po = 64 * hh
                                    nc.tensor.matmul(
                                        sps[hh][:, j * 512:(j + 1) * 512],
                                        kT[t][po:po + 64,
                                              kc * 128:(kc + 1) * 128],
                                        qT[t][po:po + 64, :],
                                        start=True, stop=True)
                            for hh in range(2):
                                pT = ppool.tile([128, 1024], bf16, tag="pT")
                                nc.scalar.activation(pT[:], sps[hh][:],
                                                     AF.Exp, scale=0.125)
                                if prev[hh] is not None:
                                    emit_pv(hh, kp - 1, prev[hh])
                                prev[hh] = pT
                        for hh in range(2):
                            emit_pv(hh, 7, prev[hh])
                        # stash rowsum rows (same-partition copies) +
                        # unnorm. attn (out-partition shift legal for
                        # 1-input copies)
                        for hh in range(2):
                            h, po = 2 * t + hh, 64 * hh
                            nc.vector.tensor_copy(
                                rs_row[64:65, h * 512:(h + 1) * 512],
                                oaccs[hh][64:65, :])
                            nc.vector.tensor_copy(aT[t][po:po + 64, :],
                                                  oaccs[hh][0:64, :])
                        # reciprocal batches: heads 0-13 after t=6 (the
                        # bounce + recip hide inside t=7's exp window),
                        # heads 14-15 at the tail.
                        if t in (6, 7):
                            gb = 0 if t == 6 else 1
                            nh = 14 if t == 6 else 2
                            c0 = 0 if t == 6 else 14 * 512
                            nc.sync.dma_start(rs_dram[gb][:],
                                              rs_row[64:65, c0:c0 + nh * 512])
                            nc.sync.dma_start(
                                rsS[gb][:],
                                rs_dram[gb].rearrange("a (p c) -> (a p) c",
                                                      p=nh))
                            with nc.allow_low_precision(
                                    reason="softmax 1/rowsum in bf16"):
                                nc.vector.reciprocal(rinvS[gb][:], rsS[gb][:])
                            nc.sync.dma_start(rinv_dram[gb][:], rinvS[gb][:])
                            for i in range(2):
                                nc.sync.dma_start(
                                    rinvA[32 * i:32 * i + 1,
                                          (c0 // 2):(c0 // 2) + (nh // 2) * 512
                                          ].rearrange("a (g c) -> a g c",
                                                      c=512),
                                    rinv_dram[gb].rearrange(
                                        "(g i) c -> i g c", i=2)[i:i + 1])
                # ---- out-proj + residual + LayerNorm ----
                # (psA/psO closed -> PSUM free for psF + psB.)
                # All 16 head-normalizes run here, interleaved with the
                # out-proj kd accumulation so the PE never waits: head
                # pair 2t,2t+1 normalizes, then kd=t accumulates.
                with tc.tile_pool(name="wop", bufs=1) as wop, \
                     tc.tile_pool(name="fin", bufs=2) as fin, \
                     tc.tile_pool(name="psB", bufs=2, space="PSUM") as psB, \
                     tc.tile_pool(name="psF", bufs=4, space="PSUM") as psF:

                    def normalize_pair(t):
                        # both heads' 1/rowsum broadcasts via col-tiled PE
                        # outer products into ONE PSUM tile; Act (idle
                        # post-Exp, Identity stays in the exp table set)
                        # evacuates so the DVE multiply runs in 2x bf16
                        # mode as a single [128,512] op.
                        bc = psB.tile([128, 512], f32, tag="bc")
                        for hh in range(2):
                            po = 64 * hh
                            nc.tensor.matmul(
                                bc[po:po + 64, :],
                                onesA[32 * hh:32 * hh + 1, :],
                                rinvA[32 * hh:32 * hh + 1,
                                      t * 512:(t + 1) * 512],
                                start=True, stop=True)
                        bcs = tmp.tile([128, 512], bf16, tag="bcs")
                        nc.scalar.activation(bcs[:], bc[:], AF.Identity)
                        nc.vector.tensor_tensor(out=aT[t][:], in0=aT[t][:],
                                                in1=bcs[:], op=ALU.mult)
                    wo_sb = [wop.tile([128, DM], bf16, name=f"wo{k}",
                                      tag=f"wo{k}") for k in range(8)]
                    for k in range(8):
                        nc.sync.dma_start(wo_sb[k][:],
                                          wo[k * 128:(k + 1) * 128, :])
                    g_sb = wop.tile([128, DM], bf16, tag="g")
                    b_sb = wop.tile([128, DM], bf16, tag="b")
                    nc.sync.dma_start(g_sb[:], gbc[:])
                    nc.sync.dma_start(b_sb[:], bbc[:])

                    def outproj(psf, mrs, kds):
                        for kd in kds:
                            for mr in mrs:
                                rr = mr * 128
                                for ncol in range(2):
                                    c0 = ncol * 512
                                    nc.tensor.matmul(
                                        psf[(mr, ncol)][:],
                                        aT[kd][:, rr:rr + 128],
                                        wo_sb[kd][:, c0:c0 + 512],
                                        start=(kd == 0), stop=(kd == 7))

                    def ln_rows(psf, mr):
                        rr = mr * 128
                        xb = fin.tile([128, DM], f32, tag="xb")
                        nc.sync.dma_start(xb[:], xr[rr:rr + 128, :])
                        # LN: one bn_stats pass for mean+var; the (h-mu)*
                        # rstd affine runs on the (post-Exp idle) Act
                        # engine as a single Identity with per-partition
                        # scale/bias.  bf16 intermediates: abs error ~2e-3
                        # of a ~5.0-max output, well inside tolerance.
                        hrow = fin.tile([128, DM], bf16, tag="hrow")
                        for ncol in range(2):
                            c0 = ncol * 512
                            nc.vector.tensor_tensor(
                                out=hrow[:, c0:c0 + 512],
                                in0=psf[(mr, ncol)][:],
                                in1=xb[:, c0:c0 + 512], op=ALU.add)
                        stats = fin.tile([128, 12], f32, tag="stats")
                        for g in range(2):
                            nc.vector.bn_stats(
                                stats[:, 6 * g:6 * g + 6],
                                hrow[:, 512 * g:512 * g + 512])
                        mv = fin.tile([128, 2], f32, tag="mv")
                        nc.vector.bn_aggr(mv[:], stats[:])
                        st = fin.tile([128, 1], f32, tag="st")
                        nc.scalar.activation(st[:], mv[:, 1:2], AF.Sqrt,
                                             bias=eps_sb[:])
                        rstd = fin.tile([128, 1], f32, tag="rstd")
                        nc.vector.reciprocal(rstd[:], st[:])
                        nmr = fin.tile([128, 1], f32, tag="nmr")
                        nc.vector.tensor_scalar(out=nmr[:], in0=mv[:, 0:1],
                                                scalar1=rstd[:],
                                                scalar2=-1.0, op0=ALU.mult,
                                                op1=ALU.mult)
                        y = fin.tile([128, DM], bf16, tag="y")
                        nc.scalar.activation(y[:], hrow[:], AF.Identity,
                                             bias=nmr[:], scale=rstd[:])
                        y2 = fin.tile([128, DM], bf16, tag="y2")
                        nc.vector.tensor_tensor(out=y2[:], in0=y[:],
                                                in1=g_sb[:], op=ALU.mult)
                        yf = fin.tile([128, DM], f32, tag="yf")
                        nc.vector.tensor_tensor(out=yf[:], in0=y2[:],
                                                in1=b_sb[:], op=ALU.add)
                        nc.sync.dma_start(out[rr:rr + 128, :], yf[:])

                    # 4 psF tags x 1 buf = 4 banks; half 2 reuses half 1's
                    # slots (waits on the half-1 LN PSUM reads).
                    psf1 = {(mr, ncol): psF.tile([128, 512], f32, bufs=1,
                                                 tag=f"f{mr}_{ncol}",
                                                 name=f"psfa{mr}_{ncol}")
                            for mr in (0, 1) for ncol in (0, 1)}
                    for t in range(8):
                        normalize_pair(t)
                        outproj(psf1, (0, 1), (t,))
                    for mr in (0, 1):
                        ln_rows(psf1, mr)
                    # emitted after the half-1 LN reads so the slot-reuse
                    # anti-dependency edges are recorded.
                    psf2 = {(mr, ncol): psF.tile([128, 512], f32, bufs=1,
                                                 tag=f"f{mr - 2}_{ncol}",
                                                 name=f"psfb{mr}_{ncol}")
                            for mr in (2, 3) for ncol in (0, 1)}
                    outproj(psf2, (2, 3), range(8))
                    for mr in (2, 3):
                        ln_rows(psf2, mr)

    _split_excess_waits(nc)
    return nc


_NC_CACHE = None


def _perm():
    p = np.zeros(DM, np.int64)
    for h in range(H):
        p[h * D:h * D + 32] = h * D + np.arange(0, D, 2)
        p[h * D + 32:(h + 1) * D] = h * D + np.arange(1, D, 2)
    return p


def kernel(x, Wqkv, bqkv, Wo, bo, gamma, beta):
    global _NC_CACHE
    x = np.asarray(x, np.float32)
    Wqkv = np.asarray(Wqkv, np.float32)
    bqkv = np.asarray(bqkv, np.float32)
    Wo = np.asarray(Wo, np.float32)
    bo = np.asarray(bo, np.float32)
    gamma = np.asarray(gamma, np.float32)
    beta = np.asarray(beta, np.float32)

    perm = _perm()
    Wq = np.ascontiguousarray(Wqkv[:, 0:DM][:, perm]).astype(BF16)
    Wk = np.ascontiguousarray(Wqkv[:, DM:2 * DM][:, perm]).astype(BF16)
    Wv = np.ascontiguousarray(Wqkv[:, 2 * DM:3 * DM]).astype(BF16)
    Wob = Wo.astype(BF16)
    bq = bqkv[0:DM][perm]
    bk = bqkv[DM:2 * DM][perm]
    bv = bqkv[2 * DM:3 * DM]

    inv = 1.0 / (10000.0 ** (np.arange(0, D, 2, dtype=np.float64) / D))
    pos = np.arange(S, dtype=np.float64)
    fr = pos[None, :] * inv[:, None]                    # [32, S]
    c32, s32 = np.cos(fr), np.sin(fr)
    CC = np.concatenate([c32, c32, c32, c32], 0).astype(BF16)   # [128, S]
    SS = np.concatenate([-s32, s32, -s32, s32], 0).astype(BF16)

    def colmajor(v):
        return np.ascontiguousarray(v.reshape(8, 128).T).astype(np.float32)

    gB = np.ascontiguousarray(np.broadcast_to(gamma, (128, DM))).astype(BF16)
    bB = np.ascontiguousarray(np.broadcast_to(beta, (128, DM))).astype(BF16)
    # bv contributes bv @ Wo to every output row (softmax weights sum to 1);
    # fold it, with bo, into the residual rows on host.
    rbias = bo + bv @ Wo

    if _NC_CACHE is None:
        _NC_CACHE = _build_program()
    nc = _NC_CACHE

    in_maps = []
    for c in range(NC):
        b, r = c // 4, c % 4
        xTb = np.ascontiguousarray(x[b].T).astype(BF16)
        rr = r * ROWS
        in_maps.append({
            "xT": xTb,
            "xTq": np.ascontiguousarray(xTb[:, rr:rr + ROWS]),
            "xr": np.ascontiguousarray(x[b, rr:rr + ROWS, :] + rbias[None, :]),
            "wq": Wq, "wk": Wk, "wv": Wv, "wo": Wob,
            "ccr": np.ascontiguousarray(CC[:, rr:rr + ROWS]),
            "ssr": np.ascontiguousarray(SS[:, rr:rr + ROWS]),
            "bqp": colmajor(bq), "bkp": colmajor(bk),
            "gbc": gB, "bbc": bB,
        })

    res = run_bass_kernel_spmd(nc, in_maps, core_ids=list(range(NC)))
    kernel._last_results = res
    full = np.empty((B, S, DM), np.float32)
    for c in range(NC):
        b, r = c // 4, c % 4
        full[b, r * ROWS:(r + 1) * ROWS, :] = res.results[c]["out"]
    return full
